# revision 24
# baseline (speedup 1.0000x reference)
"""BiLSTM-CRF forward+Viterbi Trainium2 kernel (8-core data-parallel).

Computes, for feats [S,B,T] f32, mask [S,B] f32, transitions [T,T] f32:
  logZ [B] f32         -- CRF forward log-partition
  best_score [B] f32   -- Viterbi max score
  pointers [S,B,T] i32 -- Viterbi argmax backpointers (first-max ties, exact)

Strategy (per core, B_loc=128 batches on SBUF partitions):
  Viterbi (exact fp32, bitwise-matching the jax reference):
    upd[b,i,j] = s[b,j] + trans[i,j]        (DVE tensor_tensor, broadcast AP)
    best = segmented reduce_max over j      (DVE tensor_reduce axis=X)
    eq   = (upd >= best)                    (DVE is_ge, bf16 out)
    wsel = eq * (31-j)                      (DVE bf16 2x)
    maxw = segmented reduce_max over j      -> ptr = 31 - maxw (ScalarE, i32 out)
    s    = mask ? best+feat : s             (DVE add + copy_predicated)
  Forward in linear space u = C*exp(alpha), tag-major [T,128]:
    v = exp(trans) @ u                      (PE matmul)
    u = mask ? v*exp(feat) : u              (DVE mult + copy_predicated)
    renorm every 8 steps: u /= sum(u); logacc += log(sum)   (PE+DVE+ACT)
  feats are DMA-streamed in 16-step chunks; transposed on PE for the
  tag-major exp(feat); pointers accumulated per chunk and DMA'd out.
"""

import os
import numpy as np

S_FULL, B_TOT, T, NCORES = 1024, 1024, 32, 8
B = B_TOT // NCORES
START_TAG, STOP_TAG, PAD_TAG, NEG_INF = 29, 30, 31, -10000.0

_NC_CACHE = {}
_CUSTOM_OP = None


def _get_custom_op():
    """Register SEG_ARGMAX_W_ANT: out = (in0 >= in1) * ((c0 - Idx) + c1*SubIdx).

    With in0 = upd [P, (g i) pages, j inner], c0=31, c1=32(=j count): the g/i
    page terms cancel and out = (upd >= best) * (31 - j) for every step g in
    the batch. A reduce_max over j then yields 31 - argmax_j with first-max
    (lowest-j) tie semantics, matching jnp.argmax exactly.
    """
    global _CUSTOM_OP
    if _CUSTOM_OP is not None:
        return _CUSTOM_OP
    import concourse.dve_ops as dops
    from concourse.dve_spec import Spec, Src0, Src1, C0, C1, SubIdx, Idx, lower
    from concourse.dve_uop import DveOpSpec

    name = "SEG_ARGMAX_W_ANT"
    if name in dops.CUSTOM_DVE_SPECS:
        _CUSTOM_OP = next(o for o in dops.OPS if o.name == name)
        return _CUSTOM_OP

    def _ref(in0, in1, c0, c1, c2):
        x = np.asarray(in0, dtype=np.float32)
        y = np.broadcast_to(np.asarray(in1, dtype=np.float32), x.shape)
        P = x.shape[0]
        N = x.shape[-1]
        flat = x.reshape(P, -1)
        n = flat.shape[1]
        idx = np.arange(n, dtype=np.float32)
        page = np.float32(np.arange(n) // N)
        w = (np.float32(c0) - idx) + np.float32(c1) * page
        out = (flat >= y.reshape(P, -1)).astype(np.float32) * w[None, :]
        return out.reshape(x.shape).astype(np.float32)

    spec = Spec(body=(Src0 >= Src1) * ((C0 - Idx) + C1 * SubIdx), reference=_ref)
    row = dops._CUSTOM_DVE_ROW_BASE + len(dops.OPS)
    assert row < 0x20, "custom DVE row table full"
    shas = {}
    for ver in ("v3", "v4"):
        uops = lower(spec, ver=ver)
        shas[ver] = DveOpSpec(name=name, opcode=row, uops=uops, rd1_en=True).sha(ver)
    op = dops.DveOp(name, spec, subdim=True, uops_sha=shas)
    dops.OPS.append(op)
    dops.CUSTOM_DVE_SPECS[name] = spec
    dops._SUB_OPCODE_FOR_NAME[name] = row
    _CUSTOM_OP = op
    return op


def build_nc(S, mask_from, kf=16, renorm=8, b=B, use_custom=True):
    """Build the per-core Bass program (identical on all cores)."""
    import concourse.bass as bass
    import concourse.bacc as bacc
    import concourse.mybir as mybir
    import concourse.tile as tile
    from concourse import masks
    from concourse.mybir import AluOpType as alu
    from contextlib import ExitStack

    f32 = mybir.dt.float32
    bf16 = mybir.dt.bfloat16
    i32 = mybir.dt.int32
    AX = mybir.AxisListType
    AF = mybir.ActivationFunctionType

    assert S % kf == 0 and mask_from % kf == 0
    assert kf % 4 == 0

    cop = _get_custom_op() if use_custom else None

    nc = bacc.Bacc("TRN2", target_bir_lowering=False, debug=False)

    feats_d = nc.declare_dram_parameter("feats", [S, b, T], f32, isOutput=False)
    mask_d = nc.declare_dram_parameter("mask", [S, b], f32, isOutput=False)
    transr_d = nc.declare_dram_parameter("trans_rep", [128, T * T], f32, isOutput=False)
    transTe_d = nc.declare_dram_parameter("transT_exp", [T, T], f32, isOutput=False)
    tstopr_d = nc.declare_dram_parameter("tstop_rep", [128, T], f32, isOutput=False)
    estop_d = nc.declare_dram_parameter("estop_col", [T, 1], f32, isOutput=False)
    wrep_d = nc.declare_dram_parameter("w_rep", [128, T * T], f32, isOutput=False)
    u0_d = nc.declare_dram_parameter("u0", [T, 128], f32, isOutput=False)
    ident_d = nc.declare_dram_parameter("ident128", [128, 128], f32, isOutput=False)

    logz_d = nc.declare_dram_parameter("logZ", [b], f32, isOutput=True)
    best_d = nc.declare_dram_parameter("best_score", [b], f32, isOutput=True)
    ptr_d = nc.declare_dram_parameter("pointers", [S, b, T], i32, isOutput=True)

    with tile.TileContext(nc) as tc, ExitStack() as ctx:
        # ---------------- pools ----------------
        consts = ctx.enter_context(tc.tile_pool(name="consts", bufs=1))
        state = ctx.enter_context(tc.tile_pool(name="state", bufs=1))
        fpool = ctx.enter_context(tc.tile_pool(name="fpool", bufs=2))
        ppool = ctx.enter_context(tc.tile_pool(name="ppool", bufs=2))
        vit = ctx.enter_context(tc.tile_pool(name="vit", bufs=2))
        fwd = ctx.enter_context(tc.tile_pool(name="fwd", bufs=2))
        mrow = ctx.enter_context(tc.tile_pool(name="mrow", bufs=2))
        ps_v = ctx.enter_context(tc.tile_pool(name="ps_v", bufs=2, space="PSUM"))
        ps_tr = ctx.enter_context(tc.tile_pool(name="ps_tr", bufs=2, space="PSUM"))
        ps_m = ctx.enter_context(tc.tile_pool(name="ps_m", bufs=2, space="PSUM"))
        ps_s = ctx.enter_context(tc.tile_pool(name="ps_s", bufs=1, space="PSUM"))
        ps_r = ctx.enter_context(tc.tile_pool(name="ps_r", bufs=1, space="PSUM"))

        # ---------------- constants ----------------
        ident = consts.tile([128, 128], f32)
        nc.sync.dma_start(ident[:], ident_d.ap())

        trans_rep = consts.tile([128, T, T], f32)
        nc.sync.dma_start(trans_rep[:], transr_d.ap())

        if not use_custom:
            w_rep_f = consts.tile([128, T, T], f32)
            nc.sync.dma_start(w_rep_f[:], wrep_d.ap())
            w_rep = consts.tile([128, T, T], bf16)
            nc.vector.tensor_copy(w_rep[:], w_rep_f[:])

        tstop_rep = consts.tile([128, T], f32)
        nc.sync.dma_start(tstop_rep[:], tstopr_d.ap())

        et_sb = consts.tile([T, T], f32)
        nc.sync.dma_start(et_sb[:], transTe_d.ap())
        estop_sb = consts.tile([T, 1], f32)
        nc.sync.dma_start(estop_sb[:], estop_d.ap())

        ones_1xT = consts.tile([1, T], f32)
        nc.vector.memset(ones_1xT[:], 1.0)
        ones_Tx1 = consts.tile([T, 1], f32)
        nc.vector.memset(ones_Tx1[:], 1.0)

        # ---------------- persistent state ----------------
        s_bm = state.tile([128, T], f32)  # Viterbi scores, batch-major
        nc.vector.memset(s_bm[:], NEG_INF)
        nc.vector.memset(s_bm[:, START_TAG : START_TAG + 1], 0.0)

        u_tm = state.tile([T, 128], f32)  # forward linear state, tag-major
        nc.sync.dma_start(u_tm[:], u0_d.ap())

        logacc = state.tile([1, 128], f32)
        nc.vector.memset(logacc[:], 0.0)

        maskT_sb = state.tile([128, 128], f32)  # [b, s-within-128-chunk]

        # ---------------- main loop ----------------
        n_chunks = S // kf
        for c in range(n_chunks):
            s0 = c * kf
            masked = s0 >= mask_from

            feats_bm = fpool.tile([128, kf, T], f32, name=f"feats_bm_{c}", tag="feats_bm")
            nc.sync.dma_start(feats_bm[:], feats_d.ap()[s0 : s0 + kf, :, :].transpose([1, 0, 2]))

            ptrc = ppool.tile([128, kf, T], i32, name=f"ptrc_{c}", tag="ptrc")

            if s0 % 128 == 0 and s0 + 128 > mask_from:
                # batch-major mask block for the next up-to-128 steps
                blk = min(128, S - s0)
                mk_sp = mrow.tile([128, 128], f32, name=f"mk_sp_{c}", tag="mk_sp")
                nc.sync.dma_start(mk_sp[:blk, :], mask_d.ap()[s0 : s0 + blk, :])
                mk_ps = ps_tr.tile([128, 128], f32, name=f"mk_ps_{c}", tag="tr128")
                nc.tensor.transpose(mk_ps[:, :blk], mk_sp[:blk, :], ident[:blk, :blk])
                nc.scalar.copy(maskT_sb[:, :blk], mk_ps[:, :blk])
            if masked:
                maskrow = mrow.tile([1, kf, 128], f32, name=f"maskrow_{c}", tag="maskrow")
                nc.sync.dma_start(maskrow[:], mask_d.ap()[s0 : s0 + kf, :].unsqueeze(0))

            for q in range(kf // 4):
                # transpose 4 steps of feats to tag-major [128=(s,i), 128=b];
                # ftr_ps then accumulates ln(v) per step (exp-log forward).
                ftr_ps = ps_tr.tile([128, 128], f32, name=f"ftr_{c}_{q}", tag="tr128")
                nc.tensor.transpose(ftr_ps[:], feats_bm[:, 4 * q : 4 * q + 4, :], ident[:])

                mask_tm = None
                if masked:
                    mask_tm = ps_m.tile([T, 4, 128], f32, name=f"mask_tm_{c}_{q}", tag="mask_tm")
                    nc.tensor.matmul(
                        mask_tm[:], ones_1xT[:], maskrow[:, 4 * q : 4 * q + 4, :]
                    )

                if use_custom and q % 2 == 0:
                    upd8 = vit.tile([128, 8, T, T], f32, name=f"upd8_{c}_{q}", tag="upd8")
                    best8 = vit.tile([128, 8, T], f32, name=f"best8_{c}_{q}", tag="best8")

                for dt in range(4):
                    t = s0 + 4 * q + dt
                    feat_t = feats_bm[:, 4 * q + dt, :]

                    # ---- Viterbi ----
                    if use_custom:
                        g8 = 4 * (q % 2) + dt
                        upd = upd8[:, g8]
                        bestt = best8[:, g8]
                    else:
                        upd = vit.tile([128, T, T], f32, name=f"upd_{t}", tag="upd")[:]
                        bestt = vit.tile([128, T], f32, name=f"best_{t}", tag="best")[:]
                    nc.vector.tensor_tensor(
                        upd,
                        s_bm[:].unsqueeze(1).broadcast_to([128, T, T]),
                        trans_rep[:],
                        alu.add,
                    )
                    nc.vector.reduce_max(bestt, upd, axis=AX.X)

                    if not use_custom:
                        eq = vit.tile([128, T, T], bf16, name=f"eq_{t}", tag="eq")
                        nc.vector.tensor_tensor(
                            eq[:],
                            upd,
                            bestt.unsqueeze(2).broadcast_to([128, T, T]),
                            alu.is_ge,
                        )
                        wsl = vit.tile([128, T, T], bf16, name=f"wsl_{t}", tag="wsl")
                        nc.vector.tensor_tensor(wsl[:], eq[:], w_rep[:], alu.mult)
                        maxw = vit.tile([128, T], f32, name=f"maxw_{t}", tag="maxw")
                        nc.vector.reduce_max(maxw[:], wsl[:], axis=AX.X)
                        # ptr = 31 - maxw, cast to i32 (ScalarE)
                        nc.scalar.activation(
                            ptrc[:, 4 * q + dt, :], maxw[:], AF.Copy, bias=31.0, scale=-1.0
                        )

                    # ---- scores update ----
                    if not masked:
                        nc.vector.tensor_tensor(s_bm[:], bestt, feat_t, alu.add)
                    else:
                        upd_s = vit.tile([128, T], f32, name=f"upds_{t}", tag="upds")
                        nc.vector.tensor_tensor(upd_s[:], bestt, feat_t, alu.add)
                        mcol = (
                            maskT_sb[:, t % 128 : t % 128 + 1]
                            .bitcast(mybir.dt.int32)
                            .broadcast_to([128, T])
                        )
                        nc.vector.copy_predicated(s_bm[:], mcol, upd_s[:])

                    # ---- forward: u' = exp(featT + ln(E @ u)), add on PE ----
                    vps = ps_v.tile([T, 128], f32, name=f"v_{t}", tag="v")
                    nc.tensor.matmul(vps[:], et_sb[:], u_tm[:])
                    lnv = fwd.tile([T, 128], f32, name=f"lnv_{t}", tag="lnv")
                    nc.scalar.activation(lnv[:], vps[:], AF.Ln)
                    fslice = ftr_ps[32 * dt : 32 * dt + 32, :]
                    nc.tensor.matmul(
                        fslice,
                        ident[:T, :T],
                        lnv[:],
                        start=False,
                        stop=True,
                        skip_group_check=True,
                        tile_position=(0, 32 * dt),
                    )
                    if not masked:
                        nc.scalar.activation(u_tm[:], fslice, AF.Exp)
                    else:
                        unew = fwd.tile([T, 128], f32, name=f"unew_{t}", tag="unew")
                        nc.scalar.activation(unew[:], fslice, AF.Exp)
                        nc.vector.copy_predicated(
                            u_tm[:], mask_tm[:, dt, :].bitcast(mybir.dt.int32), unew[:]
                        )

                    # ---- renorm ----
                    if (t + 1) % renorm == 0:
                        sb_ps = ps_s.tile([1, 128], f32, name=f"sb_{t}", tag="sb")
                        nc.tensor.matmul(sb_ps[:], ones_Tx1[:], u_tm[:])
                        recip = fwd.tile([1, 128], f32, name=f"recip_{t}", tag="recip")
                        nc.vector.reciprocal(recip[:], sb_ps[:])
                        lg = fwd.tile([1, 128], f32, name=f"lg_{t}", tag="lg")
                        nc.scalar.activation(lg[:], sb_ps[:], AF.Ln)
                        nc.vector.tensor_tensor(logacc[:], logacc[:], lg[:], alu.add)
                        rb_ps = ps_r.tile([T, 128], f32, name=f"rb_{t}", tag="rb")
                        nc.tensor.matmul(rb_ps[:], ones_1xT[:], recip[:])
                        nc.vector.tensor_tensor(u_tm[:], u_tm[:], rb_ps[:], alu.mult)

                if use_custom and q % 2 == 1:
                    # fused (upd >= best) * (31 - j) over the 8-step batch
                    wsl8 = vit.tile(
                        [128, 8, T, T], f32, name=f"wsl8_{c}_{q}", tag="wsl8", bufs=1
                    )
                    nc.vector._custom_dve(
                        cop,
                        out=wsl8[:].rearrange("p g a b -> p (g a) b"),
                        in0=upd8[:].rearrange("p g a b -> p (g a) b"),
                        in1=best8[:]
                        .rearrange("p g a -> p (g a)")
                        .unsqueeze(2)
                        .broadcast_to([128, 8 * T, T]),
                        s0=31.0,
                        s1=float(T),
                    )
                    maxw8 = vit.tile([128, 8, T], f32, name=f"maxw8_{c}_{q}", tag="maxw8")
                    nc.vector.reduce_max(maxw8[:], wsl8[:], axis=AX.X)
                    nc.scalar.activation(
                        ptrc[:, 4 * q - 4 : 4 * q + 4, :],
                        maxw8[:],
                        AF.Copy,
                        bias=31.0,
                        scale=-1.0,
                    )

            nc.sync.dma_start(
                ptr_d.ap()[s0 : s0 + kf, :, :].transpose([1, 0, 2]), ptrc[:]
            )

        # ---------------- endgame ----------------
        zps = ps_s.tile([1, 128], f32, name="zps", tag="sb")
        nc.tensor.matmul(zps[:], estop_sb[:], u_tm[:])
        lnz = fwd.tile([1, 128], f32, name="lnz", tag="lg")
        nc.scalar.activation(lnz[:], zps[:], AF.Ln)
        logz_t = fwd.tile([1, 128], f32, name="logz_t", tag="recip")
        nc.vector.tensor_tensor(logz_t[:], logacc[:], lnz[:], alu.add)
        nc.sync.dma_start(logz_d.ap().unsqueeze(0), logz_t[:])

        sc = vit.tile([128, T], f32, name="sc", tag="upds")
        nc.vector.tensor_tensor(sc[:], s_bm[:], tstop_rep[:], alu.add)
        bsc = vit.tile([128, 1], f32, name="bsc", tag="maxw")
        nc.vector.reduce_max(bsc[:], sc[:], axis=AX.X)
        nc.sync.dma_start(best_d.ap().unsqueeze(1), bsc[:])

    nc.compile()
    return nc


def _prep_small_inputs(transitions):
    f32 = np.float32
    tr = np.asarray(transitions, dtype=f32)
    trans_rep = np.ascontiguousarray(
        np.broadcast_to(tr.reshape(1, T * T), (128, T * T))
    )
    e = np.exp(np.maximum(tr, f32(-87.0))).astype(f32)
    transT_exp = np.ascontiguousarray(e.T)
    tstop_rep = np.ascontiguousarray(np.broadcast_to(tr[STOP_TAG], (128, T)))
    estop_col = np.ascontiguousarray(e[STOP_TAG].reshape(T, 1))
    w_rep = np.ascontiguousarray(
        np.broadcast_to(
            np.tile((31 - np.arange(T)).astype(f32), T).reshape(1, T * T),
            (128, T * T),
        )
    )
    u0 = np.zeros((T, 128), dtype=f32)
    u0[START_TAG, :] = 1.0
    return {
        "u0": u0,
        "trans_rep": trans_rep,
        "transT_exp": transT_exp,
        "tstop_rep": tstop_rep,
        "estop_col": estop_col,
        "w_rep": w_rep,
        "ident128": np.eye(128, dtype=f32),
    }


def _get_nc(S, mask_from):
    use_custom = os.environ.get("CRF_NO_CUSTOM", "0") != "1"
    key = (S, mask_from, use_custom)
    if key not in _NC_CACHE:
        _NC_CACHE[key] = build_nc(S, mask_from, use_custom=use_custom)
    return _NC_CACHE[key]


def _install_trace_support():
    """Synthesize the missing antenv.axon_hooks module + disable artifact upload."""
    import sys, types

    if "antenv.axon_hooks" not in sys.modules:
        mod = types.ModuleType("antenv.axon_hooks")
        mod._hook = None

        def set_axon_ntff_profile_hook(h):
            mod._hook = h

        def get_axon_ntff_profile_hook():
            return mod._hook

        mod.set_axon_ntff_profile_hook = set_axon_ntff_profile_hook
        mod.get_axon_ntff_profile_hook = get_axon_ntff_profile_hook
        sys.modules["antenv.axon_hooks"] = mod
        try:
            import antenv

            antenv.axon_hooks = mod
        except Exception:
            pass
    m = sys.modules["antenv.axon_hooks"]
    if m._hook is None:
        try:
            from trn_agent_boot.trn_boot import _ntff_profile_via_ctypes

            m.set_axon_ntff_profile_hook(
                _ntff_profile_via_ctypes("/opt/axon/libaxon_pjrt.so")
            )
        except Exception as e:
            print(f"ntff hook install failed: {e}")
    import concourse.bass_utils as bu

    if not getattr(bu, "_upload_patched", False):
        bu.upload_artifacts = lambda tmpdir: f"local:{tmpdir}"
        bu._upload_patched = True


def kernel_with_results(feats, mask, transitions, trace=False):
    from concourse.bass_utils import run_bass_kernel_spmd

    if trace:
        _install_trace_support()

    feats = np.asarray(feats, dtype=np.float32)
    mask = np.asarray(mask, dtype=np.float32)
    S, Btot, Tt = feats.shape
    assert Tt == T and Btot % NCORES == 0
    b = Btot // NCORES
    assert b == B

    lens = mask.sum(axis=0)
    kf = 16
    mask_from = int(min(lens.min() // kf * kf, S))
    nc = _get_nc(S, mask_from)

    small = _prep_small_inputs(transitions)
    in_maps = []
    for c in range(NCORES):
        sl = slice(c * b, (c + 1) * b)
        in_maps.append(
            {
                "feats": np.ascontiguousarray(feats[:, sl, :]),
                "mask": np.ascontiguousarray(mask[:, sl]),
                **small,
            }
        )

    import tempfile

    tmpdir = tempfile.mkdtemp(prefix="crf_trace_") if trace else None
    res = run_bass_kernel_spmd(
        nc, in_maps, list(range(NCORES)), trace=trace, tmpdir=tmpdir
    )
    if trace:
        print(f"trace dir: {tmpdir}")
    outs = res.results
    logZ = np.concatenate([np.asarray(o["logZ"]).reshape(-1) for o in outs])
    best = np.concatenate([np.asarray(o["best_score"]).reshape(-1) for o in outs])
    ptr = np.concatenate(
        [np.asarray(o["pointers"]).reshape(S, b, T) for o in outs], axis=1
    )
    return (logZ.astype(np.float32), best.astype(np.float32), ptr.astype(np.int32)), res


def kernel(feats, mask, transitions):
    (logZ, best, ptr), _ = kernel_with_results(feats, mask, transitions, trace=False)
    return logZ, best, ptr


# revision 26
# speedup vs baseline: 1.2788x; 1.2788x over previous
"""BiLSTM-CRF forward+Viterbi Trainium2 kernel (8-core data-parallel).

Computes, for feats [S,B,T] f32, mask [S,B] f32, transitions [T,T] f32:
  logZ [B] f32         -- CRF forward log-partition
  best_score [B] f32   -- Viterbi max score
  pointers [S,B,T] i32 -- Viterbi argmax backpointers (first-max ties, exact)

Strategy (per core, B_loc=128 batches on SBUF partitions):
  Viterbi (exact fp32, bitwise-matching the jax reference):
    upd[b,i,j] = s[b,j] + trans[i,j]        (DVE tensor_tensor, broadcast AP)
    best = segmented reduce_max over j      (DVE tensor_reduce axis=X)
    eq   = (upd >= best)                    (DVE is_ge, bf16 out)
    wsel = eq * (31-j)                      (DVE bf16 2x)
    maxw = segmented reduce_max over j      -> ptr = 31 - maxw (ScalarE, i32 out)
    s    = mask ? best+feat : s             (DVE add + copy_predicated)
  Forward in linear space u = C*exp(alpha), tag-major [T,128]:
    v = exp(trans) @ u                      (PE matmul)
    u = mask ? v*exp(feat) : u              (DVE mult + copy_predicated)
    renorm every 8 steps: u /= sum(u); logacc += log(sum)   (PE+DVE+ACT)
  feats are DMA-streamed in 16-step chunks; transposed on PE for the
  tag-major exp(feat); pointers accumulated per chunk and DMA'd out.
"""

import os
import numpy as np

S_FULL, B_TOT, T, NCORES = 1024, 1024, 32, 8
B = B_TOT // NCORES
START_TAG, STOP_TAG, PAD_TAG, NEG_INF = 29, 30, 31, -10000.0

_NC_CACHE = {}
_CUSTOM_OP = None


def _get_custom_op():
    """Register SEG_ARGMAX_W_ANT: out = (in0 >= in1) * ((c0 - Idx) + c1*SubIdx).

    With in0 = upd [P, (g i) pages, j inner], c0=31, c1=32(=j count): the g/i
    page terms cancel and out = (upd >= best) * (31 - j) for every step g in
    the batch. A reduce_max over j then yields 31 - argmax_j with first-max
    (lowest-j) tie semantics, matching jnp.argmax exactly.
    """
    global _CUSTOM_OP
    if _CUSTOM_OP is not None:
        return _CUSTOM_OP
    import concourse.dve_ops as dops
    from concourse.dve_spec import Spec, Src0, Src1, C0, C1, SubIdx, Idx, lower
    from concourse.dve_uop import DveOpSpec

    name = "SEG_ARGMAX_W_ANT"
    if name in dops.CUSTOM_DVE_SPECS:
        _CUSTOM_OP = next(o for o in dops.OPS if o.name == name)
        return _CUSTOM_OP

    def _ref(in0, in1, c0, c1, c2):
        x = np.asarray(in0, dtype=np.float32)
        y = np.broadcast_to(np.asarray(in1, dtype=np.float32), x.shape)
        P = x.shape[0]
        N = x.shape[-1]
        flat = x.reshape(P, -1)
        n = flat.shape[1]
        idx = np.arange(n, dtype=np.float32)
        page = np.float32(np.arange(n) // N)
        w = (np.float32(c0) - idx) + np.float32(c1) * page
        out = (flat >= y.reshape(P, -1)).astype(np.float32) * w[None, :]
        return out.reshape(x.shape).astype(np.float32)

    spec = Spec(body=(Src0 >= Src1) * ((C0 - Idx) + C1 * SubIdx), reference=_ref)
    row = dops._CUSTOM_DVE_ROW_BASE + len(dops.OPS)
    assert row < 0x20, "custom DVE row table full"
    shas = {}
    for ver in ("v3", "v4"):
        uops = lower(spec, ver=ver)
        shas[ver] = DveOpSpec(name=name, opcode=row, uops=uops, rd1_en=True).sha(ver)
    op = dops.DveOp(name, spec, subdim=True, uops_sha=shas)
    dops.OPS.append(op)
    dops.CUSTOM_DVE_SPECS[name] = spec
    dops._SUB_OPCODE_FOR_NAME[name] = row
    _CUSTOM_OP = op
    return op


def build_nc(S, mask_from, kf=16, renorm=8, b=B, use_custom=True):
    """Build the per-core Bass program (identical on all cores)."""
    import concourse.bass as bass
    import concourse.bacc as bacc
    import concourse.mybir as mybir
    import concourse.tile as tile
    from concourse import masks
    from concourse.mybir import AluOpType as alu
    from contextlib import ExitStack

    f32 = mybir.dt.float32
    bf16 = mybir.dt.bfloat16
    i32 = mybir.dt.int32
    AX = mybir.AxisListType
    AF = mybir.ActivationFunctionType

    assert S % kf == 0 and mask_from % kf == 0
    assert kf % 4 == 0

    cop = _get_custom_op() if use_custom else None

    nc = bacc.Bacc("TRN2", target_bir_lowering=False, debug=False)

    feats_d = nc.declare_dram_parameter("feats", [S, b, T], f32, isOutput=False)
    mask_d = nc.declare_dram_parameter("mask", [S, b], f32, isOutput=False)
    transr_d = nc.declare_dram_parameter("trans_rep", [128, T * T], f32, isOutput=False)
    transTe_d = nc.declare_dram_parameter("transT_exp", [T, T], f32, isOutput=False)
    tstopr_d = nc.declare_dram_parameter("tstop_rep", [128, T], f32, isOutput=False)
    estop_d = nc.declare_dram_parameter("estop_col", [T, 1], f32, isOutput=False)
    wrep_d = nc.declare_dram_parameter("w_rep", [128, T * T], f32, isOutput=False)
    u0_d = nc.declare_dram_parameter("u0", [T, 128], f32, isOutput=False)
    ident_d = nc.declare_dram_parameter("ident128", [128, 128], f32, isOutput=False)

    logz_d = nc.declare_dram_parameter("logZ", [b], f32, isOutput=True)
    best_d = nc.declare_dram_parameter("best_score", [b], f32, isOutput=True)
    ptr_d = nc.declare_dram_parameter("pointers", [S, b, T], i32, isOutput=True)

    with tile.TileContext(nc) as tc, ExitStack() as ctx:
        # ---------------- pools ----------------
        consts = ctx.enter_context(tc.tile_pool(name="consts", bufs=1))
        state = ctx.enter_context(tc.tile_pool(name="state", bufs=1))
        fpool = ctx.enter_context(tc.tile_pool(name="fpool", bufs=2))
        ppool = ctx.enter_context(tc.tile_pool(name="ppool", bufs=2))
        vit = ctx.enter_context(tc.tile_pool(name="vit", bufs=2))
        fwd = ctx.enter_context(tc.tile_pool(name="fwd", bufs=2))
        mrow = ctx.enter_context(tc.tile_pool(name="mrow", bufs=2))
        ps_v = ctx.enter_context(tc.tile_pool(name="ps_v", bufs=2, space="PSUM"))
        ps_tr = ctx.enter_context(tc.tile_pool(name="ps_tr", bufs=2, space="PSUM"))
        ps_m = ctx.enter_context(tc.tile_pool(name="ps_m", bufs=2, space="PSUM"))
        ps_s = ctx.enter_context(tc.tile_pool(name="ps_s", bufs=1, space="PSUM"))
        ps_r = ctx.enter_context(tc.tile_pool(name="ps_r", bufs=1, space="PSUM"))

        # ---------------- constants ----------------
        ident = consts.tile([128, 128], f32)
        nc.sync.dma_start(ident[:], ident_d.ap())

        trans_rep = consts.tile([128, T, T], f32)
        nc.sync.dma_start(trans_rep[:], transr_d.ap())

        if not use_custom:
            w_rep_f = consts.tile([128, T, T], f32)
            nc.sync.dma_start(w_rep_f[:], wrep_d.ap())
            w_rep = consts.tile([128, T, T], bf16)
            nc.vector.tensor_copy(w_rep[:], w_rep_f[:])

        tstop_rep = consts.tile([128, T], f32)
        nc.sync.dma_start(tstop_rep[:], tstopr_d.ap())

        et_sb = consts.tile([T, T], f32)
        nc.sync.dma_start(et_sb[:], transTe_d.ap())
        estop_sb = consts.tile([T, 1], f32)
        nc.sync.dma_start(estop_sb[:], estop_d.ap())

        ones_1xT = consts.tile([1, T], f32)
        nc.vector.memset(ones_1xT[:], 1.0)
        ones_Tx1 = consts.tile([T, 1], f32)
        nc.vector.memset(ones_Tx1[:], 1.0)

        # ---------------- persistent state ----------------
        s_bm = state.tile([128, T], f32)  # Viterbi scores, batch-major
        nc.vector.memset(s_bm[:], NEG_INF)
        nc.vector.memset(s_bm[:, START_TAG : START_TAG + 1], 0.0)

        u_tm = state.tile([T, 128], f32)  # forward linear state, tag-major
        nc.sync.dma_start(u_tm[:], u0_d.ap())

        logacc = state.tile([1, 128], f32)
        nc.vector.memset(logacc[:], 0.0)

        maskT_sb = state.tile([128, 128], f32)  # [b, s-within-128-chunk]

        # ---------------- main loop ----------------
        n_chunks = S // kf
        for c in range(n_chunks):
            s0 = c * kf
            masked = s0 >= mask_from

            feats_bm = fpool.tile([128, kf, T], f32, name=f"feats_bm_{c}", tag="feats_bm")
            nc.sync.dma_start(feats_bm[:], feats_d.ap()[s0 : s0 + kf, :, :].transpose([1, 0, 2]))

            ptrc = ppool.tile([128, kf, T], i32, name=f"ptrc_{c}", tag="ptrc")

            if s0 % 128 == 0 and s0 + 128 > mask_from:
                # batch-major mask block for the next up-to-128 steps
                blk = min(128, S - s0)
                mk_sp = mrow.tile([128, 128], f32, name=f"mk_sp_{c}", tag="mk_sp")
                nc.sync.dma_start(mk_sp[:blk, :], mask_d.ap()[s0 : s0 + blk, :])
                mk_ps = ps_tr.tile([128, 128], f32, name=f"mk_ps_{c}", tag="tr128")
                nc.tensor.transpose(mk_ps[:, :blk], mk_sp[:blk, :], ident[:blk, :blk])
                nc.scalar.copy(maskT_sb[:, :blk], mk_ps[:, :blk])
            if masked:
                maskrow = mrow.tile([1, kf, 128], f32, name=f"maskrow_{c}", tag="maskrow")
                nc.sync.dma_start(maskrow[:], mask_d.ap()[s0 : s0 + kf, :].unsqueeze(0))

            for q in range(kf // 4):
                # transpose 4 steps of feats to tag-major, then exp -> g4
                ftr_ps = ps_tr.tile([128, 128], f32, name=f"ftr_{c}_{q}", tag="tr128")
                nc.tensor.transpose(ftr_ps[:], feats_bm[:, 4 * q : 4 * q + 4, :], ident[:])
                g4 = fwd.tile([128, 128], f32, name=f"g4_{c}_{q}", tag="g4")
                nc.scalar.activation(g4[:], ftr_ps[:], AF.Exp)

                mask_tm = None
                if masked:
                    mask_tm = ps_m.tile([T, 4, 128], f32, name=f"mask_tm_{c}_{q}", tag="mask_tm")
                    nc.tensor.matmul(
                        mask_tm[:], ones_1xT[:], maskrow[:, 4 * q : 4 * q + 4, :]
                    )

                if use_custom and q % 2 == 0:
                    upd8 = vit.tile([128, 8, T, T], f32, name=f"upd8_{c}_{q}", tag="upd8")
                    best8 = vit.tile([128, 8, T], f32, name=f"best8_{c}_{q}", tag="best8")

                for dt in range(4):
                    t = s0 + 4 * q + dt
                    feat_t = feats_bm[:, 4 * q + dt, :]

                    # ---- Viterbi ----
                    if use_custom:
                        g8 = 4 * (q % 2) + dt
                        upd = upd8[:, g8]
                        bestt = best8[:, g8]
                    else:
                        upd = vit.tile([128, T, T], f32, name=f"upd_{t}", tag="upd")[:]
                        bestt = vit.tile([128, T], f32, name=f"best_{t}", tag="best")[:]
                    nc.vector.tensor_tensor(
                        upd,
                        s_bm[:].unsqueeze(1).broadcast_to([128, T, T]),
                        trans_rep[:],
                        alu.add,
                    )
                    nc.vector.reduce_max(bestt, upd, axis=AX.X)

                    if not use_custom:
                        eq = vit.tile([128, T, T], bf16, name=f"eq_{t}", tag="eq")
                        nc.vector.tensor_tensor(
                            eq[:],
                            upd,
                            bestt.unsqueeze(2).broadcast_to([128, T, T]),
                            alu.is_ge,
                        )
                        wsl = vit.tile([128, T, T], bf16, name=f"wsl_{t}", tag="wsl")
                        nc.vector.tensor_tensor(wsl[:], eq[:], w_rep[:], alu.mult)
                        maxw = vit.tile([128, T], f32, name=f"maxw_{t}", tag="maxw")
                        nc.vector.reduce_max(maxw[:], wsl[:], axis=AX.X)
                        # ptr = 31 - maxw, cast to i32 (ScalarE)
                        nc.scalar.activation(
                            ptrc[:, 4 * q + dt, :], maxw[:], AF.Copy, bias=31.0, scale=-1.0
                        )

                    # ---- scores update ----
                    if not masked:
                        nc.vector.tensor_tensor(s_bm[:], bestt, feat_t, alu.add)
                    else:
                        upd_s = vit.tile([128, T], f32, name=f"upds_{t}", tag="upds")
                        nc.vector.tensor_tensor(upd_s[:], bestt, feat_t, alu.add)
                        mcol = (
                            maskT_sb[:, t % 128 : t % 128 + 1]
                            .bitcast(mybir.dt.int32)
                            .broadcast_to([128, T])
                        )
                        nc.vector.copy_predicated(s_bm[:], mcol, upd_s[:])

                    # ---- forward ----
                    vps = ps_v.tile([T, 128], f32, name=f"v_{t}", tag="v")
                    nc.tensor.matmul(vps[:], et_sb[:], u_tm[:])
                    g_t = g4[32 * dt : 32 * dt + 32, :]
                    if not masked:
                        nc.vector.tensor_tensor(u_tm[:], vps[:], g_t, alu.mult)
                    else:
                        unew = fwd.tile([T, 128], f32, name=f"unew_{t}", tag="unew")
                        nc.vector.tensor_tensor(unew[:], vps[:], g_t, alu.mult)
                        nc.vector.copy_predicated(
                            u_tm[:], mask_tm[:, dt, :].bitcast(mybir.dt.int32), unew[:]
                        )

                    # ---- renorm ----
                    if (t + 1) % renorm == 0:
                        sb_ps = ps_s.tile([1, 128], f32, name=f"sb_{t}", tag="sb")
                        nc.tensor.matmul(sb_ps[:], ones_Tx1[:], u_tm[:])
                        recip = fwd.tile([1, 128], f32, name=f"recip_{t}", tag="recip")
                        nc.vector.reciprocal(recip[:], sb_ps[:])
                        lg = fwd.tile([1, 128], f32, name=f"lg_{t}", tag="lg")
                        nc.scalar.activation(lg[:], sb_ps[:], AF.Ln)
                        nc.vector.tensor_tensor(logacc[:], logacc[:], lg[:], alu.add)
                        rb_ps = ps_r.tile([T, 128], f32, name=f"rb_{t}", tag="rb")
                        nc.tensor.matmul(rb_ps[:], ones_1xT[:], recip[:])
                        nc.vector.tensor_tensor(u_tm[:], u_tm[:], rb_ps[:], alu.mult)

                if use_custom and q % 2 == 1:
                    # fused (upd >= best) * (31 - j) over the 8-step batch
                    wsl8 = vit.tile(
                        [128, 8, T, T], f32, name=f"wsl8_{c}_{q}", tag="wsl8", bufs=1
                    )
                    nc.vector._custom_dve(
                        cop,
                        out=wsl8[:].rearrange("p g a b -> p (g a) b"),
                        in0=upd8[:].rearrange("p g a b -> p (g a) b"),
                        in1=best8[:]
                        .rearrange("p g a -> p (g a)")
                        .unsqueeze(2)
                        .broadcast_to([128, 8 * T, T]),
                        s0=31.0,
                        s1=float(T),
                    )
                    maxw8 = vit.tile([128, 8, T], f32, name=f"maxw8_{c}_{q}", tag="maxw8")
                    nc.vector.reduce_max(maxw8[:], wsl8[:], axis=AX.X)
                    nc.scalar.activation(
                        ptrc[:, 4 * q - 4 : 4 * q + 4, :],
                        maxw8[:],
                        AF.Copy,
                        bias=31.0,
                        scale=-1.0,
                    )

            nc.sync.dma_start(
                ptr_d.ap()[s0 : s0 + kf, :, :].transpose([1, 0, 2]), ptrc[:]
            )

        # ---------------- endgame ----------------
        zps = ps_s.tile([1, 128], f32, name="zps", tag="sb")
        nc.tensor.matmul(zps[:], estop_sb[:], u_tm[:])
        lnz = fwd.tile([1, 128], f32, name="lnz", tag="lg")
        nc.scalar.activation(lnz[:], zps[:], AF.Ln)
        logz_t = fwd.tile([1, 128], f32, name="logz_t", tag="recip")
        nc.vector.tensor_tensor(logz_t[:], logacc[:], lnz[:], alu.add)
        nc.sync.dma_start(logz_d.ap().unsqueeze(0), logz_t[:])

        sc = vit.tile([128, T], f32, name="sc", tag="upds")
        nc.vector.tensor_tensor(sc[:], s_bm[:], tstop_rep[:], alu.add)
        bsc = vit.tile([128, 1], f32, name="bsc", tag="maxw")
        nc.vector.reduce_max(bsc[:], sc[:], axis=AX.X)
        nc.sync.dma_start(best_d.ap().unsqueeze(1), bsc[:])

    nc.compile()
    return nc


def _prep_small_inputs(transitions):
    f32 = np.float32
    tr = np.asarray(transitions, dtype=f32)
    trans_rep = np.ascontiguousarray(
        np.broadcast_to(tr.reshape(1, T * T), (128, T * T))
    )
    e = np.exp(np.maximum(tr, f32(-87.0))).astype(f32)
    transT_exp = np.ascontiguousarray(e.T)
    tstop_rep = np.ascontiguousarray(np.broadcast_to(tr[STOP_TAG], (128, T)))
    estop_col = np.ascontiguousarray(e[STOP_TAG].reshape(T, 1))
    w_rep = np.ascontiguousarray(
        np.broadcast_to(
            np.tile((31 - np.arange(T)).astype(f32), T).reshape(1, T * T),
            (128, T * T),
        )
    )
    u0 = np.zeros((T, 128), dtype=f32)
    u0[START_TAG, :] = 1.0
    return {
        "u0": u0,
        "trans_rep": trans_rep,
        "transT_exp": transT_exp,
        "tstop_rep": tstop_rep,
        "estop_col": estop_col,
        "w_rep": w_rep,
        "ident128": np.eye(128, dtype=f32),
    }


def _get_nc(S, mask_from):
    use_custom = os.environ.get("CRF_NO_CUSTOM", "0") != "1"
    key = (S, mask_from, use_custom)
    if key not in _NC_CACHE:
        _NC_CACHE[key] = build_nc(S, mask_from, use_custom=use_custom)
    return _NC_CACHE[key]


def _install_trace_support():
    """Synthesize the missing antenv.axon_hooks module + disable artifact upload."""
    import sys, types

    if "antenv.axon_hooks" not in sys.modules:
        mod = types.ModuleType("antenv.axon_hooks")
        mod._hook = None

        def set_axon_ntff_profile_hook(h):
            mod._hook = h

        def get_axon_ntff_profile_hook():
            return mod._hook

        mod.set_axon_ntff_profile_hook = set_axon_ntff_profile_hook
        mod.get_axon_ntff_profile_hook = get_axon_ntff_profile_hook
        sys.modules["antenv.axon_hooks"] = mod
        try:
            import antenv

            antenv.axon_hooks = mod
        except Exception:
            pass
    m = sys.modules["antenv.axon_hooks"]
    if m._hook is None:
        try:
            from trn_agent_boot.trn_boot import _ntff_profile_via_ctypes

            m.set_axon_ntff_profile_hook(
                _ntff_profile_via_ctypes("/opt/axon/libaxon_pjrt.so")
            )
        except Exception as e:
            print(f"ntff hook install failed: {e}")
    import concourse.bass_utils as bu

    if not getattr(bu, "_upload_patched", False):
        bu.upload_artifacts = lambda tmpdir: f"local:{tmpdir}"
        bu._upload_patched = True


def kernel_with_results(feats, mask, transitions, trace=False):
    from concourse.bass_utils import run_bass_kernel_spmd

    if trace:
        _install_trace_support()

    feats = np.asarray(feats, dtype=np.float32)
    mask = np.asarray(mask, dtype=np.float32)
    S, Btot, Tt = feats.shape
    assert Tt == T and Btot % NCORES == 0
    b = Btot // NCORES
    assert b == B

    lens = mask.sum(axis=0)
    kf = 16
    mask_from = int(min(lens.min() // kf * kf, S))
    nc = _get_nc(S, mask_from)

    small = _prep_small_inputs(transitions)
    in_maps = []
    for c in range(NCORES):
        sl = slice(c * b, (c + 1) * b)
        in_maps.append(
            {
                "feats": np.ascontiguousarray(feats[:, sl, :]),
                "mask": np.ascontiguousarray(mask[:, sl]),
                **small,
            }
        )

    import tempfile

    tmpdir = tempfile.mkdtemp(prefix="crf_trace_") if trace else None
    res = run_bass_kernel_spmd(
        nc, in_maps, list(range(NCORES)), trace=trace, tmpdir=tmpdir
    )
    if trace:
        print(f"trace dir: {tmpdir}")
    outs = res.results
    logZ = np.concatenate([np.asarray(o["logZ"]).reshape(-1) for o in outs])
    best = np.concatenate([np.asarray(o["best_score"]).reshape(-1) for o in outs])
    ptr = np.concatenate(
        [np.asarray(o["pointers"]).reshape(S, b, T) for o in outs], axis=1
    )
    return (logZ.astype(np.float32), best.astype(np.float32), ptr.astype(np.int32)), res


def kernel(feats, mask, transitions):
    (logZ, best, ptr), _ = kernel_with_results(feats, mask, transitions, trace=False)
    return logZ, best, ptr


# revision 28
# speedup vs baseline: 1.2789x; 1.0001x over previous
"""BiLSTM-CRF forward+Viterbi Trainium2 kernel (8-core data-parallel).

Computes, for feats [S,B,T] f32, mask [S,B] f32, transitions [T,T] f32:
  logZ [B] f32         -- CRF forward log-partition
  best_score [B] f32   -- Viterbi max score
  pointers [S,B,T] i32 -- Viterbi argmax backpointers (first-max ties, exact)

Strategy (per core, B_loc=128 batches on SBUF partitions):
  Viterbi (exact fp32, bitwise-matching the jax reference):
    upd[b,i,j] = s[b,j] + trans[i,j]        (DVE tensor_tensor, broadcast AP)
    best = segmented reduce_max over j      (DVE tensor_reduce axis=X)
    wsel = (upd >= best) * (31-j)           (custom DVE op, 8-step batched)
    maxw = segmented reduce_max over j      -> ptr = 31 - maxw (ScalarE, i32 out)
    s    = mask ? best+feat : s             (DVE add + copy_predicated)
  The wsel/maxw extraction reproduces jnp.argmax first-max tie semantics
  exactly (best is the reduce_max of the same fl(s+t) values).
  Forward in linear space u = C*exp(alpha), tag-major [T,128]:
    v = exp(trans) @ u                      (PE matmul)
    u = mask ? v*exp(feat) : u              (DVE mult + copy_predicated)
    renorm every 8 steps: u /= sum(u); logacc += log(sum)   (PE+DVE+ACT)
  feats are DMA-streamed in 16-step chunks; transposed on PE for the
  tag-major exp(feat); pointers accumulated per chunk and DMA'd out.
"""

import os
import numpy as np

S_FULL, B_TOT, T, NCORES = 1024, 1024, 32, 8
B = B_TOT // NCORES
START_TAG, STOP_TAG, PAD_TAG, NEG_INF = 29, 30, 31, -10000.0

_NC_CACHE = {}
_CUSTOM_OP = None


def _get_custom_op():
    """Register SEG_ARGMAX_W_ANT: out = (in0 >= in1) * ((c0 - Idx) + c1*SubIdx).

    With in0 = upd [P, (g i) pages, j inner], c0=31, c1=32(=j count): the g/i
    page terms cancel and out = (upd >= best) * (31 - j) for every step g in
    the batch. A reduce_max over j then yields 31 - argmax_j with first-max
    (lowest-j) tie semantics, matching jnp.argmax exactly.
    """
    global _CUSTOM_OP
    if _CUSTOM_OP is not None:
        return _CUSTOM_OP
    import concourse.dve_ops as dops
    from concourse.dve_spec import Spec, Src0, Src1, C0, C1, SubIdx, Idx, lower
    from concourse.dve_uop import DveOpSpec

    name = "SEG_ARGMAX_W_ANT"
    if name in dops.CUSTOM_DVE_SPECS:
        _CUSTOM_OP = next(o for o in dops.OPS if o.name == name)
        return _CUSTOM_OP

    def _ref(in0, in1, c0, c1, c2):
        x = np.asarray(in0, dtype=np.float32)
        y = np.broadcast_to(np.asarray(in1, dtype=np.float32), x.shape)
        P = x.shape[0]
        N = x.shape[-1]
        flat = x.reshape(P, -1)
        n = flat.shape[1]
        idx = np.arange(n, dtype=np.float32)
        page = np.float32(np.arange(n) // N)
        w = (np.float32(c0) - idx) + np.float32(c1) * page
        out = (flat >= y.reshape(P, -1)).astype(np.float32) * w[None, :]
        return out.reshape(x.shape).astype(np.float32)

    spec = Spec(body=(Src0 >= Src1) * ((C0 - Idx) + C1 * SubIdx), reference=_ref)
    row = dops._CUSTOM_DVE_ROW_BASE + len(dops.OPS)
    assert row < 0x20, "custom DVE row table full"
    shas = {}
    for ver in ("v3", "v4"):
        uops = lower(spec, ver=ver)
        shas[ver] = DveOpSpec(name=name, opcode=row, uops=uops, rd1_en=True).sha(ver)
    op = dops.DveOp(name, spec, subdim=True, uops_sha=shas)
    dops.OPS.append(op)
    dops.CUSTOM_DVE_SPECS[name] = spec
    dops._SUB_OPCODE_FOR_NAME[name] = row
    _CUSTOM_OP = op
    return op


def build_nc(S, mask_from, kf=16, renorm=8, b=B, use_custom=True):
    """Build the per-core Bass program (identical on all cores)."""
    import concourse.bass as bass
    import concourse.bacc as bacc
    import concourse.mybir as mybir
    import concourse.tile as tile
    from concourse.mybir import AluOpType as alu
    from contextlib import ExitStack

    f32 = mybir.dt.float32
    bf16 = mybir.dt.bfloat16
    i32 = mybir.dt.int32
    AX = mybir.AxisListType
    AF = mybir.ActivationFunctionType

    assert S % kf == 0 and mask_from % kf == 0
    assert kf % 4 == 0

    cop = _get_custom_op() if use_custom else None

    nc = bacc.Bacc("TRN2", target_bir_lowering=False, debug=False)

    feats_d = nc.declare_dram_parameter("feats", [S, b, T], f32, isOutput=False)
    mask_d = nc.declare_dram_parameter("mask", [S, b], f32, isOutput=False)
    transr_d = nc.declare_dram_parameter("trans_rep", [128, T * T], f32, isOutput=False)
    transTe_d = nc.declare_dram_parameter("transT_exp", [T, T], f32, isOutput=False)
    tstopr_d = nc.declare_dram_parameter("tstop_rep", [128, T], f32, isOutput=False)
    estop_d = nc.declare_dram_parameter("estop_col", [T, 1], f32, isOutput=False)
    wrep_d = nc.declare_dram_parameter("w_rep", [128, T * T], f32, isOutput=False)
    u0_d = nc.declare_dram_parameter("u0", [T, 128], f32, isOutput=False)
    ident_d = nc.declare_dram_parameter("ident128", [128, 128], f32, isOutput=False)

    logz_d = nc.declare_dram_parameter("logZ", [b], f32, isOutput=True)
    best_d = nc.declare_dram_parameter("best_score", [b], f32, isOutput=True)
    ptr_d = nc.declare_dram_parameter("pointers", [S, b, T], i32, isOutput=True)

    with tile.TileContext(nc) as tc, ExitStack() as ctx:
        # ---------------- pools ----------------
        consts = ctx.enter_context(tc.tile_pool(name="consts", bufs=1))
        state = ctx.enter_context(tc.tile_pool(name="state", bufs=1))
        fpool = ctx.enter_context(tc.tile_pool(name="fpool", bufs=2))
        ppool = ctx.enter_context(tc.tile_pool(name="ppool", bufs=2))
        vit = ctx.enter_context(tc.tile_pool(name="vit", bufs=2))
        fwd = ctx.enter_context(tc.tile_pool(name="fwd", bufs=2))
        mrow = ctx.enter_context(tc.tile_pool(name="mrow", bufs=2))
        ps_v = ctx.enter_context(tc.tile_pool(name="ps_v", bufs=2, space="PSUM"))
        ps_tr = ctx.enter_context(tc.tile_pool(name="ps_tr", bufs=2, space="PSUM"))
        ps_m = ctx.enter_context(tc.tile_pool(name="ps_m", bufs=2, space="PSUM"))
        ps_s = ctx.enter_context(tc.tile_pool(name="ps_s", bufs=1, space="PSUM"))
        ps_r = ctx.enter_context(tc.tile_pool(name="ps_r", bufs=1, space="PSUM"))

        # ---------------- constants ----------------
        ident = consts.tile([128, 128], f32)
        nc.sync.dma_start(ident[:], ident_d.ap())

        trans_rep = consts.tile([128, T, T], f32)
        nc.sync.dma_start(trans_rep[:], transr_d.ap())

        if not use_custom:
            w_rep_f = consts.tile([128, T, T], f32)
            nc.sync.dma_start(w_rep_f[:], wrep_d.ap())
            w_rep = consts.tile([128, T, T], bf16)
            nc.vector.tensor_copy(w_rep[:], w_rep_f[:])

        tstop_rep = consts.tile([128, T], f32)
        nc.sync.dma_start(tstop_rep[:], tstopr_d.ap())

        et_sb = consts.tile([T, T], f32)
        nc.sync.dma_start(et_sb[:], transTe_d.ap())
        estop_sb = consts.tile([T, 1], f32)
        nc.sync.dma_start(estop_sb[:], estop_d.ap())

        ones_1xT = consts.tile([1, T], f32)
        nc.vector.memset(ones_1xT[:], 1.0)
        ones_Tx1 = consts.tile([T, 1], f32)
        nc.vector.memset(ones_Tx1[:], 1.0)

        # ---------------- persistent state ----------------
        s_bm = state.tile([128, T], f32)  # Viterbi scores, batch-major
        nc.vector.memset(s_bm[:], NEG_INF)
        nc.vector.memset(s_bm[:, START_TAG : START_TAG + 1], 0.0)

        u_tm = state.tile([T, 128], f32)  # forward linear state, tag-major
        nc.sync.dma_start(u_tm[:], u0_d.ap())

        logacc = state.tile([1, 128], f32)
        nc.vector.memset(logacc[:], 0.0)

        maskT_sb = state.tile([128, 128], f32)  # [b, s-within-128-chunk]

        # ---------------- main loop ----------------
        n_chunks = S // kf
        for c in range(n_chunks):
            s0 = c * kf
            masked = s0 >= mask_from

            feats_bm = fpool.tile([128, kf, T], f32, name=f"feats_bm_{c}", tag="feats_bm")
            nc.sync.dma_start(feats_bm[:], feats_d.ap()[s0 : s0 + kf, :, :].transpose([1, 0, 2]))

            ptrc = ppool.tile([128, kf, T], i32, name=f"ptrc_{c}", tag="ptrc")

            if s0 % 128 == 0 and s0 + 128 > mask_from:
                # batch-major mask block for the next up-to-128 steps
                blk = min(128, S - s0)
                mk_sp = mrow.tile([128, 128], f32, name=f"mk_sp_{c}", tag="mk_sp")
                nc.sync.dma_start(mk_sp[:blk, :], mask_d.ap()[s0 : s0 + blk, :])
                mk_ps = ps_tr.tile([128, 128], f32, name=f"mk_ps_{c}", tag="tr128")
                nc.tensor.transpose(mk_ps[:, :blk], mk_sp[:blk, :], ident[:blk, :blk])
                nc.scalar.copy(maskT_sb[:, :blk], mk_ps[:, :blk])
            if masked:
                maskrow = mrow.tile([1, kf, 128], f32, name=f"maskrow_{c}", tag="maskrow")
                nc.sync.dma_start(maskrow[:], mask_d.ap()[s0 : s0 + kf, :].unsqueeze(0))

            for q in range(kf // 4):
                # transpose 4 steps of feats to tag-major, then exp -> g4
                ftr_ps = ps_tr.tile([128, 128], f32, name=f"ftr_{c}_{q}", tag="tr128")
                nc.tensor.transpose(ftr_ps[:], feats_bm[:, 4 * q : 4 * q + 4, :], ident[:])
                g4 = fwd.tile([128, 128], f32, name=f"g4_{c}_{q}", tag="g4")
                nc.scalar.activation(g4[:], ftr_ps[:], AF.Exp)

                mask_tm = None
                if masked:
                    mask_tm = ps_m.tile([T, 4, 128], f32, name=f"mask_tm_{c}_{q}", tag="mask_tm")
                    nc.tensor.matmul(
                        mask_tm[:], ones_1xT[:], maskrow[:, 4 * q : 4 * q + 4, :]
                    )

                if use_custom and q % 2 == 0:
                    upd8 = vit.tile([128, 8, T, T], f32, name=f"upd8_{c}_{q}", tag="upd8")
                    best8 = vit.tile([128, 8, T], f32, name=f"best8_{c}_{q}", tag="best8")

                for dt in range(4):
                    t = s0 + 4 * q + dt
                    feat_t = feats_bm[:, 4 * q + dt, :]

                    # ---- Viterbi ----
                    if use_custom:
                        g8 = 4 * (q % 2) + dt
                        upd = upd8[:, g8]
                        bestt = best8[:, g8]
                    else:
                        upd = vit.tile([128, T, T], f32, name=f"upd_{t}", tag="upd")[:]
                        bestt = vit.tile([128, T], f32, name=f"best_{t}", tag="best")[:]
                    nc.vector.tensor_tensor(
                        upd,
                        s_bm[:].unsqueeze(1).broadcast_to([128, T, T]),
                        trans_rep[:],
                        alu.add,
                    )
                    nc.vector.reduce_max(bestt, upd, axis=AX.X)

                    if not use_custom:
                        eq = vit.tile([128, T, T], bf16, name=f"eq_{t}", tag="eq")
                        nc.vector.tensor_tensor(
                            eq[:],
                            upd,
                            bestt.unsqueeze(2).broadcast_to([128, T, T]),
                            alu.is_ge,
                        )
                        wsl = vit.tile([128, T, T], bf16, name=f"wsl_{t}", tag="wsl")
                        nc.vector.tensor_tensor(wsl[:], eq[:], w_rep[:], alu.mult)
                        maxw = vit.tile([128, T], f32, name=f"maxw_{t}", tag="maxw")
                        nc.vector.reduce_max(maxw[:], wsl[:], axis=AX.X)
                        # ptr = 31 - maxw, cast to i32 (ScalarE)
                        nc.scalar.activation(
                            ptrc[:, 4 * q + dt, :], maxw[:], AF.Copy, bias=31.0, scale=-1.0
                        )

                    # ---- scores update ----
                    if not masked:
                        nc.vector.tensor_tensor(s_bm[:], bestt, feat_t, alu.add)
                    else:
                        upd_s = vit.tile([128, T], f32, name=f"upds_{t}", tag="upds")
                        nc.vector.tensor_tensor(upd_s[:], bestt, feat_t, alu.add)
                        mcol = (
                            maskT_sb[:, t % 128 : t % 128 + 1]
                            .bitcast(mybir.dt.int32)
                            .broadcast_to([128, T])
                        )
                        nc.vector.copy_predicated(s_bm[:], mcol, upd_s[:])

                    # ---- forward ----
                    vps = ps_v.tile([T, 128], f32, name=f"v_{t}", tag="v")
                    nc.tensor.matmul(vps[:], et_sb[:], u_tm[:])
                    g_t = g4[32 * dt : 32 * dt + 32, :]
                    if not masked:
                        nc.vector.tensor_tensor(u_tm[:], vps[:], g_t, alu.mult)
                    else:
                        unew = fwd.tile([T, 128], f32, name=f"unew_{t}", tag="unew")
                        nc.vector.tensor_tensor(unew[:], vps[:], g_t, alu.mult)
                        nc.vector.copy_predicated(
                            u_tm[:], mask_tm[:, dt, :].bitcast(mybir.dt.int32), unew[:]
                        )

                    # ---- renorm ----
                    if (t + 1) % renorm == 0:
                        sb_ps = ps_s.tile([1, 128], f32, name=f"sb_{t}", tag="sb")
                        nc.tensor.matmul(sb_ps[:], ones_Tx1[:], u_tm[:])
                        recip = fwd.tile([1, 128], f32, name=f"recip_{t}", tag="recip")
                        nc.vector.reciprocal(recip[:], sb_ps[:])
                        lg = fwd.tile([1, 128], f32, name=f"lg_{t}", tag="lg")
                        nc.scalar.activation(lg[:], sb_ps[:], AF.Ln)
                        nc.vector.tensor_tensor(logacc[:], logacc[:], lg[:], alu.add)
                        rb_ps = ps_r.tile([T, 128], f32, name=f"rb_{t}", tag="rb")
                        nc.tensor.matmul(rb_ps[:], ones_1xT[:], recip[:])
                        nc.vector.tensor_tensor(u_tm[:], u_tm[:], rb_ps[:], alu.mult)

                if use_custom and q % 2 == 1:
                    # fused (upd >= best) * (31 - j) over the 8-step batch
                    wsl8 = vit.tile(
                        [128, 8, T, T], f32, name=f"wsl8_{c}_{q}", tag="wsl8", bufs=1
                    )
                    nc.vector._custom_dve(
                        cop,
                        out=wsl8[:].rearrange("p g a b -> p (g a) b"),
                        in0=upd8[:].rearrange("p g a b -> p (g a) b"),
                        in1=best8[:]
                        .rearrange("p g a -> p (g a)")
                        .unsqueeze(2)
                        .broadcast_to([128, 8 * T, T]),
                        s0=31.0,
                        s1=float(T),
                    )
                    maxw8 = vit.tile([128, 8, T], f32, name=f"maxw8_{c}_{q}", tag="maxw8")
                    nc.vector.reduce_max(maxw8[:], wsl8[:], axis=AX.X)
                    nc.scalar.activation(
                        ptrc[:, 4 * q - 4 : 4 * q + 4, :],
                        maxw8[:],
                        AF.Copy,
                        bias=31.0,
                        scale=-1.0,
                    )

            nc.sync.dma_start(
                ptr_d.ap()[s0 : s0 + kf, :, :].transpose([1, 0, 2]), ptrc[:]
            )

        # ---------------- endgame ----------------
        zps = ps_s.tile([1, 128], f32, name="zps", tag="sb")
        nc.tensor.matmul(zps[:], estop_sb[:], u_tm[:])
        lnz = fwd.tile([1, 128], f32, name="lnz", tag="lg")
        nc.scalar.activation(lnz[:], zps[:], AF.Ln)
        logz_t = fwd.tile([1, 128], f32, name="logz_t", tag="recip")
        nc.vector.tensor_tensor(logz_t[:], logacc[:], lnz[:], alu.add)
        nc.sync.dma_start(logz_d.ap().unsqueeze(0), logz_t[:])

        sc = vit.tile([128, T], f32, name="sc", tag="upds")
        nc.vector.tensor_tensor(sc[:], s_bm[:], tstop_rep[:], alu.add)
        bsc = vit.tile([128, 1], f32, name="bsc", tag="maxw")
        nc.vector.reduce_max(bsc[:], sc[:], axis=AX.X)
        nc.sync.dma_start(best_d.ap().unsqueeze(1), bsc[:])

    nc.compile()
    return nc


def _prep_small_inputs(transitions):
    f32 = np.float32
    tr = np.asarray(transitions, dtype=f32)
    trans_rep = np.ascontiguousarray(
        np.broadcast_to(tr.reshape(1, T * T), (128, T * T))
    )
    e = np.exp(np.maximum(tr, f32(-87.0))).astype(f32)
    transT_exp = np.ascontiguousarray(e.T)
    tstop_rep = np.ascontiguousarray(np.broadcast_to(tr[STOP_TAG], (128, T)))
    estop_col = np.ascontiguousarray(e[STOP_TAG].reshape(T, 1))
    w_rep = np.ascontiguousarray(
        np.broadcast_to(
            np.tile((31 - np.arange(T)).astype(f32), T).reshape(1, T * T),
            (128, T * T),
        )
    )
    u0 = np.zeros((T, 128), dtype=f32)
    u0[START_TAG, :] = 1.0
    return {
        "u0": u0,
        "trans_rep": trans_rep,
        "transT_exp": transT_exp,
        "tstop_rep": tstop_rep,
        "estop_col": estop_col,
        "w_rep": w_rep,
        "ident128": np.eye(128, dtype=f32),
    }


def _get_nc(S, mask_from):
    use_custom = os.environ.get("CRF_NO_CUSTOM", "0") != "1"
    key = (S, mask_from, use_custom)
    if key not in _NC_CACHE:
        _NC_CACHE[key] = build_nc(S, mask_from, use_custom=use_custom)
    return _NC_CACHE[key]


def _install_trace_support():
    """Synthesize the missing antenv.axon_hooks module + disable artifact upload."""
    import sys, types

    if "antenv.axon_hooks" not in sys.modules:
        mod = types.ModuleType("antenv.axon_hooks")
        mod._hook = None

        def set_axon_ntff_profile_hook(h):
            mod._hook = h

        def get_axon_ntff_profile_hook():
            return mod._hook

        mod.set_axon_ntff_profile_hook = set_axon_ntff_profile_hook
        mod.get_axon_ntff_profile_hook = get_axon_ntff_profile_hook
        sys.modules["antenv.axon_hooks"] = mod
        try:
            import antenv

            antenv.axon_hooks = mod
        except Exception:
            pass
    m = sys.modules["antenv.axon_hooks"]
    if m._hook is None:
        try:
            from trn_agent_boot.trn_boot import _ntff_profile_via_ctypes

            m.set_axon_ntff_profile_hook(
                _ntff_profile_via_ctypes("/opt/axon/libaxon_pjrt.so")
            )
        except Exception as e:
            print(f"ntff hook install failed: {e}")
    import concourse.bass_utils as bu

    if not getattr(bu, "_upload_patched", False):
        bu.upload_artifacts = lambda tmpdir: f"local:{tmpdir}"
        bu._upload_patched = True


def kernel_with_results(feats, mask, transitions, trace=False):
    from concourse.bass_utils import run_bass_kernel_spmd

    if trace:
        _install_trace_support()

    feats = np.asarray(feats, dtype=np.float32)
    mask = np.asarray(mask, dtype=np.float32)
    S, Btot, Tt = feats.shape
    assert Tt == T and Btot % NCORES == 0
    b = Btot // NCORES
    assert b == B

    lens = mask.sum(axis=0)
    kf = 16
    mask_from = int(min(lens.min() // kf * kf, S))
    nc = _get_nc(S, mask_from)

    small = _prep_small_inputs(transitions)
    in_maps = []
    for c in range(NCORES):
        sl = slice(c * b, (c + 1) * b)
        in_maps.append(
            {
                "feats": np.ascontiguousarray(feats[:, sl, :]),
                "mask": np.ascontiguousarray(mask[:, sl]),
                **small,
            }
        )

    import tempfile

    tmpdir = tempfile.mkdtemp(prefix="crf_trace_") if trace else None
    res = run_bass_kernel_spmd(
        nc, in_maps, list(range(NCORES)), trace=trace, tmpdir=tmpdir
    )
    if trace:
        print(f"trace dir: {tmpdir}")
    outs = res.results
    logZ = np.concatenate([np.asarray(o["logZ"]).reshape(-1) for o in outs])
    best = np.concatenate([np.asarray(o["best_score"]).reshape(-1) for o in outs])
    ptr = np.concatenate(
        [np.asarray(o["pointers"]).reshape(S, b, T) for o in outs], axis=1
    )
    return (logZ.astype(np.float32), best.astype(np.float32), ptr.astype(np.int32)), res


def kernel(feats, mask, transitions):
    (logZ, best, ptr), _ = kernel_with_results(feats, mask, transitions, trace=False)
    return logZ, best, ptr


# revision 29
# speedup vs baseline: 1.4165x; 1.1076x over previous
"""BiLSTM-CRF forward+Viterbi Trainium2 kernel (8-core data-parallel).

Computes, for feats [S,B,T] f32, mask [S,B] f32, transitions [T,T] f32:
  logZ [B] f32         -- CRF forward log-partition
  best_score [B] f32   -- Viterbi max score
  pointers [S,B,T] i32 -- Viterbi argmax backpointers (first-max ties, exact)

Strategy (per core, B_loc=128 batches on SBUF partitions):
  Viterbi (exact fp32, bitwise-matching the jax reference):
    upd[b,i,j] = s[b,j] + trans[i,j]        (DVE tensor_tensor, broadcast AP)
    best = segmented reduce_max over j      (DVE tensor_reduce axis=X)
    wsel = (upd >= best) * (31-j)           (custom DVE op, 8-step batched)
    maxw = segmented reduce_max over j      -> ptr = 31 - maxw (ScalarE, i32 out)
    s    = mask ? best+feat : s             (DVE add + copy_predicated)
  The wsel/maxw extraction reproduces jnp.argmax first-max tie semantics
  exactly (best is the reduce_max of the same fl(s+t) values).
  Forward in linear space u = C*exp(alpha), tag-major [T,128]:
    v = exp(trans) @ u                      (PE matmul)
    u = mask ? v*exp(feat) : u              (DVE mult + copy_predicated)
    renorm every 8 steps: u /= sum(u); logacc += log(sum)   (PE+DVE+ACT)
  feats are DMA-streamed in 16-step chunks; transposed on PE for the
  tag-major exp(feat); pointers accumulated per chunk and DMA'd out.
"""

import os
import numpy as np

S_FULL, B_TOT, T, NCORES = 1024, 1024, 32, 8
B = B_TOT // NCORES
START_TAG, STOP_TAG, PAD_TAG, NEG_INF = 29, 30, 31, -10000.0

_NC_CACHE = {}
_CUSTOM_OP = None


def _get_custom_op():
    """Register SEG_ARGMAX_W_ANT: out = (in0 >= in1) * ((c0 - Idx) + c1*SubIdx).

    With in0 = upd [P, (g i) pages, j inner], c0=31, c1=32(=j count): the g/i
    page terms cancel and out = (upd >= best) * (31 - j) for every step g in
    the batch. A reduce_max over j then yields 31 - argmax_j with first-max
    (lowest-j) tie semantics, matching jnp.argmax exactly.
    """
    global _CUSTOM_OP
    if _CUSTOM_OP is not None:
        return _CUSTOM_OP
    import concourse.dve_ops as dops
    from concourse.dve_spec import Spec, Src0, Src1, C0, C1, SubIdx, Idx, lower
    from concourse.dve_uop import DveOpSpec

    name = "SEG_ARGMAX_W_ANT"
    if name in dops.CUSTOM_DVE_SPECS:
        _CUSTOM_OP = next(o for o in dops.OPS if o.name == name)
        return _CUSTOM_OP

    def _ref(in0, in1, c0, c1, c2):
        x = np.asarray(in0, dtype=np.float32)
        y = np.broadcast_to(np.asarray(in1, dtype=np.float32), x.shape)
        P = x.shape[0]
        N = x.shape[-1]
        flat = x.reshape(P, -1)
        n = flat.shape[1]
        idx = np.arange(n, dtype=np.float32)
        page = np.float32(np.arange(n) // N)
        w = (np.float32(c0) - idx) + np.float32(c1) * page
        out = (flat >= y.reshape(P, -1)).astype(np.float32) * w[None, :]
        return out.reshape(x.shape).astype(np.float32)

    spec = Spec(body=(Src0 >= Src1) * ((C0 - Idx) + C1 * SubIdx), reference=_ref)
    row = dops._CUSTOM_DVE_ROW_BASE + len(dops.OPS)
    assert row < 0x20, "custom DVE row table full"
    shas = {}
    for ver in ("v3", "v4"):
        uops = lower(spec, ver=ver)
        shas[ver] = DveOpSpec(name=name, opcode=row, uops=uops, rd1_en=True).sha(ver)
    op = dops.DveOp(name, spec, subdim=True, uops_sha=shas)
    dops.OPS.append(op)
    dops.CUSTOM_DVE_SPECS[name] = spec
    dops._SUB_OPCODE_FOR_NAME[name] = row
    _CUSTOM_OP = op
    return op


def build_nc(S, mask_from, kf=16, renorm=8, b=B, use_custom=True):
    """Build the per-core Bass program (identical on all cores)."""
    import concourse.bass as bass
    import concourse.bacc as bacc
    import concourse.mybir as mybir
    import concourse.tile as tile
    from concourse.mybir import AluOpType as alu
    from contextlib import ExitStack

    f32 = mybir.dt.float32
    bf16 = mybir.dt.bfloat16
    i32 = mybir.dt.int32
    AX = mybir.AxisListType
    AF = mybir.ActivationFunctionType

    assert S % kf == 0 and mask_from % kf == 0
    assert kf % 4 == 0

    cop = _get_custom_op() if use_custom else None

    nc = bacc.Bacc("TRN2", target_bir_lowering=False, debug=False)

    feats_d = nc.declare_dram_parameter("feats", [S, b, T], f32, isOutput=False)
    mask_d = nc.declare_dram_parameter("mask", [S, b], f32, isOutput=False)
    transr_d = nc.declare_dram_parameter("trans_rep", [128, T * T], f32, isOutput=False)
    transTe_d = nc.declare_dram_parameter("transT_exp", [T, T], f32, isOutput=False)
    tstopr_d = nc.declare_dram_parameter("tstop_rep", [128, T], f32, isOutput=False)
    estop_d = nc.declare_dram_parameter("estop_col", [T, 1], f32, isOutput=False)
    wrep_d = nc.declare_dram_parameter("w_rep", [128, T * T], f32, isOutput=False)
    u0_d = nc.declare_dram_parameter("u0", [T, 128], f32, isOutput=False)
    ident_d = nc.declare_dram_parameter("ident128", [128, 128], f32, isOutput=False)

    logz_d = nc.declare_dram_parameter("logZ", [b], f32, isOutput=True)
    best_d = nc.declare_dram_parameter("best_score", [b], f32, isOutput=True)
    ptr_d = nc.declare_dram_parameter("pointers", [S, b, T], i32, isOutput=True)

    with tile.TileContext(nc) as tc, ExitStack() as ctx:
        # ---------------- pools ----------------
        consts = ctx.enter_context(tc.tile_pool(name="consts", bufs=1))
        state = ctx.enter_context(tc.tile_pool(name="state", bufs=1))
        fpool = ctx.enter_context(tc.tile_pool(name="fpool", bufs=2))
        ppool = ctx.enter_context(tc.tile_pool(name="ppool", bufs=2))
        vit = ctx.enter_context(tc.tile_pool(name="vit", bufs=2))
        fwd = ctx.enter_context(tc.tile_pool(name="fwd", bufs=2))
        mrow = ctx.enter_context(tc.tile_pool(name="mrow", bufs=2))
        ps_v = ctx.enter_context(tc.tile_pool(name="ps_v", bufs=2, space="PSUM"))
        ps_tr = ctx.enter_context(tc.tile_pool(name="ps_tr", bufs=2, space="PSUM"))
        ps_m = ctx.enter_context(tc.tile_pool(name="ps_m", bufs=2, space="PSUM"))
        ps_s = ctx.enter_context(tc.tile_pool(name="ps_s", bufs=1, space="PSUM"))
        ps_r = ctx.enter_context(tc.tile_pool(name="ps_r", bufs=1, space="PSUM"))

        # ---------------- constants ----------------
        ident = consts.tile([128, 128], f32)
        nc.sync.dma_start(ident[:], ident_d.ap())

        trans_rep = consts.tile([128, T, T], f32)
        nc.sync.dma_start(trans_rep[:], transr_d.ap())

        if not use_custom:
            w_rep_f = consts.tile([128, T, T], f32)
            nc.sync.dma_start(w_rep_f[:], wrep_d.ap())
            w_rep = consts.tile([128, T, T], bf16)
            nc.vector.tensor_copy(w_rep[:], w_rep_f[:])

        tstop_rep = consts.tile([128, T], f32)
        nc.sync.dma_start(tstop_rep[:], tstopr_d.ap())

        et_sb = consts.tile([T, T], f32)
        nc.sync.dma_start(et_sb[:], transTe_d.ap())
        estop_sb = consts.tile([T, 1], f32)
        nc.sync.dma_start(estop_sb[:], estop_d.ap())

        ones_1xT = consts.tile([1, T], f32)
        nc.vector.memset(ones_1xT[:], 1.0)
        ones_Tx1 = consts.tile([T, 1], f32)
        nc.vector.memset(ones_Tx1[:], 1.0)

        # ---------------- persistent state ----------------
        s_bm = state.tile([128, T], f32)  # Viterbi scores, batch-major
        nc.vector.memset(s_bm[:], NEG_INF)
        nc.vector.memset(s_bm[:, START_TAG : START_TAG + 1], 0.0)

        u_tm = state.tile([T, 128], f32)  # forward linear state, tag-major
        nc.sync.dma_start(u_tm[:], u0_d.ap())

        logacc = state.tile([1, 128], f32)
        nc.vector.memset(logacc[:], 0.0)

        maskT_sb = state.tile([128, 128], f32)  # [b, s-within-128-chunk]

        # ---------------- main loop ----------------
        n_chunks = S // kf
        for c in range(n_chunks):
            s0 = c * kf
            masked = s0 >= mask_from

            feats_bm = fpool.tile([128, kf, T], f32, name=f"feats_bm_{c}", tag="feats_bm")
            nc.sync.dma_start(feats_bm[:], feats_d.ap()[s0 : s0 + kf, :, :].transpose([1, 0, 2]))

            ptrc = ppool.tile([128, kf, T], i32, name=f"ptrc_{c}", tag="ptrc")

            if s0 % 128 == 0 and s0 + 128 > mask_from:
                # batch-major mask block for the next up-to-128 steps
                blk = min(128, S - s0)
                mk_sp = mrow.tile([128, 128], f32, name=f"mk_sp_{c}", tag="mk_sp")
                nc.sync.dma_start(mk_sp[:blk, :], mask_d.ap()[s0 : s0 + blk, :])
                mk_ps = ps_tr.tile([128, 128], f32, name=f"mk_ps_{c}", tag="tr128")
                nc.tensor.transpose(mk_ps[:, :blk], mk_sp[:blk, :], ident[:blk, :blk])
                nc.scalar.copy(maskT_sb[:, :blk], mk_ps[:, :blk])
            if masked:
                maskrow = mrow.tile([1, kf, 128], f32, name=f"maskrow_{c}", tag="maskrow")
                nc.sync.dma_start(maskrow[:], mask_d.ap()[s0 : s0 + kf, :].unsqueeze(0))

            for q in range(kf // 4):
                # transpose 4 steps of feats to tag-major, then exp -> g4
                ftr_ps = ps_tr.tile([128, 128], f32, name=f"ftr_{c}_{q}", tag="tr128")
                nc.tensor.transpose(ftr_ps[:], feats_bm[:, 4 * q : 4 * q + 4, :], ident[:])
                g4 = fwd.tile([128, 128], f32, name=f"g4_{c}_{q}", tag="g4")
                nc.scalar.activation(g4[:], ftr_ps[:], AF.Exp)

                mask_tm = None
                if masked:
                    mask_tm = ps_m.tile([T, 4, 128], f32, name=f"mask_tm_{c}_{q}", tag="mask_tm")
                    nc.tensor.matmul(
                        mask_tm[:], ones_1xT[:], maskrow[:, 4 * q : 4 * q + 4, :]
                    )

                if use_custom and q % 2 == 0:
                    upd8 = vit.tile([128, 8, T, T], f32, name=f"upd8_{c}_{q}", tag="upd8")
                    best8 = vit.tile([128, 8, T], f32, name=f"best8_{c}_{q}", tag="best8")

                for dt in range(4):
                    t = s0 + 4 * q + dt
                    feat_t = feats_bm[:, 4 * q + dt, :]

                    # ---- Viterbi ----
                    if use_custom:
                        g8 = 4 * (q % 2) + dt
                        upd = upd8[:, g8]
                        bestt = best8[:, g8]
                    else:
                        upd = vit.tile([128, T, T], f32, name=f"upd_{t}", tag="upd")[:]
                        bestt = vit.tile([128, T], f32, name=f"best_{t}", tag="best")[:]
                    nc.vector.tensor_tensor(
                        upd,
                        s_bm[:].unsqueeze(1).broadcast_to([128, T, T]),
                        trans_rep[:],
                        alu.add,
                    )
                    nc.vector.reduce_max(bestt, upd, axis=AX.X)

                    if not use_custom:
                        eq = vit.tile([128, T, T], bf16, name=f"eq_{t}", tag="eq")
                        nc.vector.tensor_tensor(
                            eq[:],
                            upd,
                            bestt.unsqueeze(2).broadcast_to([128, T, T]),
                            alu.is_ge,
                        )
                        wsl = vit.tile([128, T, T], bf16, name=f"wsl_{t}", tag="wsl")
                        nc.vector.tensor_tensor(wsl[:], eq[:], w_rep[:], alu.mult)
                        maxw = vit.tile([128, T], f32, name=f"maxw_{t}", tag="maxw")
                        nc.vector.reduce_max(maxw[:], wsl[:], axis=AX.X)
                        # ptr = 31 - maxw, cast to i32 (ScalarE)
                        nc.scalar.activation(
                            ptrc[:, 4 * q + dt, :], maxw[:], AF.Copy, bias=31.0, scale=-1.0
                        )

                    # ---- scores update ----
                    if not masked:
                        nc.vector.tensor_tensor(s_bm[:], bestt, feat_t, alu.add)
                    else:
                        upd_s = vit.tile([128, T], f32, name=f"upds_{t}", tag="upds")
                        nc.vector.tensor_tensor(upd_s[:], bestt, feat_t, alu.add)
                        mcol = (
                            maskT_sb[:, t % 128 : t % 128 + 1]
                            .bitcast(mybir.dt.int32)
                            .broadcast_to([128, T])
                        )
                        nc.vector.copy_predicated(s_bm[:], mcol, upd_s[:])

                    # ---- forward ----
                    vps = ps_v.tile([T, 128], f32, name=f"v_{t}", tag="v")
                    nc.tensor.matmul(vps[:], et_sb[:], u_tm[:])
                    g_t = g4[32 * dt : 32 * dt + 32, :]
                    if not masked:
                        nc.vector.tensor_tensor(u_tm[:], vps[:], g_t, alu.mult)
                    else:
                        unew = fwd.tile([T, 128], f32, name=f"unew_{t}", tag="unew")
                        nc.vector.tensor_tensor(unew[:], vps[:], g_t, alu.mult)
                        nc.vector.copy_predicated(
                            u_tm[:], mask_tm[:, dt, :].bitcast(mybir.dt.int32), unew[:]
                        )

                    # ---- renorm ----
                    if (t + 1) % renorm == 0:
                        sb_ps = ps_s.tile([1, 128], f32, name=f"sb_{t}", tag="sb")
                        nc.tensor.matmul(sb_ps[:], ones_Tx1[:], u_tm[:])
                        recip = fwd.tile([1, 128], f32, name=f"recip_{t}", tag="recip")
                        nc.vector.reciprocal(recip[:], sb_ps[:])
                        lg = fwd.tile([1, 128], f32, name=f"lg_{t}", tag="lg")
                        nc.scalar.activation(lg[:], sb_ps[:], AF.Ln)
                        nc.vector.tensor_tensor(logacc[:], logacc[:], lg[:], alu.add)
                        rb_ps = ps_r.tile([T, 128], f32, name=f"rb_{t}", tag="rb")
                        nc.tensor.matmul(rb_ps[:], ones_1xT[:], recip[:])
                        nc.vector.tensor_tensor(u_tm[:], u_tm[:], rb_ps[:], alu.mult)

                if use_custom and q % 2 == 1:
                    # fused (upd >= best) * (31 - j) over the 8-step batch.
                    # wsel values are 0..31 integers -> bf16-exact, so the
                    # segmented max runs as a contiguous-halves tensor_tensor
                    # max tree in bf16 2x mode (tensor_reduce is 1x-only).
                    wsl8 = vit.tile(
                        [128, 8, T, T], bf16, name=f"wsl8_{c}_{q}", tag="wsl8", bufs=1
                    )
                    nc.vector._custom_dve(
                        cop,
                        out=wsl8[:].rearrange("p g a b -> p (g a) b"),
                        in0=upd8[:].rearrange("p g a b -> p (g a) b"),
                        in1=best8[:]
                        .rearrange("p g a -> p (g a)")
                        .unsqueeze(2)
                        .broadcast_to([128, 8 * T, T]),
                        s0=31.0,
                        s1=float(T),
                    )
                    w3 = wsl8[:].rearrange("p g a b -> p (g a) b")  # [128, 256, 32]
                    m1 = vit.tile([128, 256, 16], bf16, name=f"m1_{c}_{q}", tag="m1", bufs=1)
                    nc.vector.tensor_tensor(m1[:], w3[:, :, 0:16], w3[:, :, 16:32], alu.max)
                    m2 = vit.tile([128, 256, 8], bf16, name=f"m2_{c}_{q}", tag="m2", bufs=1)
                    nc.vector.tensor_tensor(m2[:], m1[:, :, 0:8], m1[:, :, 8:16], alu.max)
                    m3 = vit.tile([128, 256, 4], bf16, name=f"m3_{c}_{q}", tag="m3", bufs=1)
                    nc.vector.tensor_tensor(m3[:], m2[:, :, 0:4], m2[:, :, 4:8], alu.max)
                    m4 = vit.tile([128, 256, 2], bf16, name=f"m4_{c}_{q}", tag="m4", bufs=1)
                    nc.vector.tensor_tensor(m4[:], m3[:, :, 0:2], m3[:, :, 2:4], alu.max)
                    maxw8 = vit.tile([128, 8, T], bf16, name=f"maxw8_{c}_{q}", tag="maxw8")
                    nc.vector.tensor_tensor(
                        maxw8[:].rearrange("p g a -> p (g a)").unsqueeze(2),
                        m4[:, :, 0:1],
                        m4[:, :, 1:2],
                        alu.max,
                    )
                    nc.scalar.activation(
                        ptrc[:, 4 * q - 4 : 4 * q + 4, :],
                        maxw8[:],
                        AF.Copy,
                        bias=31.0,
                        scale=-1.0,
                    )

            nc.sync.dma_start(
                ptr_d.ap()[s0 : s0 + kf, :, :].transpose([1, 0, 2]), ptrc[:]
            )

        # ---------------- endgame ----------------
        zps = ps_s.tile([1, 128], f32, name="zps", tag="sb")
        nc.tensor.matmul(zps[:], estop_sb[:], u_tm[:])
        lnz = fwd.tile([1, 128], f32, name="lnz", tag="lg")
        nc.scalar.activation(lnz[:], zps[:], AF.Ln)
        logz_t = fwd.tile([1, 128], f32, name="logz_t", tag="recip")
        nc.vector.tensor_tensor(logz_t[:], logacc[:], lnz[:], alu.add)
        nc.sync.dma_start(logz_d.ap().unsqueeze(0), logz_t[:])

        sc = vit.tile([128, T], f32, name="sc", tag="upds")
        nc.vector.tensor_tensor(sc[:], s_bm[:], tstop_rep[:], alu.add)
        bsc = vit.tile([128, 1], f32, name="bsc", tag="maxw")
        nc.vector.reduce_max(bsc[:], sc[:], axis=AX.X)
        nc.sync.dma_start(best_d.ap().unsqueeze(1), bsc[:])

    nc.compile()
    return nc


def _prep_small_inputs(transitions):
    f32 = np.float32
    tr = np.asarray(transitions, dtype=f32)
    trans_rep = np.ascontiguousarray(
        np.broadcast_to(tr.reshape(1, T * T), (128, T * T))
    )
    e = np.exp(np.maximum(tr, f32(-87.0))).astype(f32)
    transT_exp = np.ascontiguousarray(e.T)
    tstop_rep = np.ascontiguousarray(np.broadcast_to(tr[STOP_TAG], (128, T)))
    estop_col = np.ascontiguousarray(e[STOP_TAG].reshape(T, 1))
    w_rep = np.ascontiguousarray(
        np.broadcast_to(
            np.tile((31 - np.arange(T)).astype(f32), T).reshape(1, T * T),
            (128, T * T),
        )
    )
    u0 = np.zeros((T, 128), dtype=f32)
    u0[START_TAG, :] = 1.0
    return {
        "u0": u0,
        "trans_rep": trans_rep,
        "transT_exp": transT_exp,
        "tstop_rep": tstop_rep,
        "estop_col": estop_col,
        "w_rep": w_rep,
        "ident128": np.eye(128, dtype=f32),
    }


def _get_nc(S, mask_from):
    use_custom = os.environ.get("CRF_NO_CUSTOM", "0") != "1"
    key = (S, mask_from, use_custom)
    if key not in _NC_CACHE:
        _NC_CACHE[key] = build_nc(S, mask_from, use_custom=use_custom)
    return _NC_CACHE[key]


def _install_trace_support():
    """Synthesize the missing antenv.axon_hooks module + disable artifact upload."""
    import sys, types

    if "antenv.axon_hooks" not in sys.modules:
        mod = types.ModuleType("antenv.axon_hooks")
        mod._hook = None

        def set_axon_ntff_profile_hook(h):
            mod._hook = h

        def get_axon_ntff_profile_hook():
            return mod._hook

        mod.set_axon_ntff_profile_hook = set_axon_ntff_profile_hook
        mod.get_axon_ntff_profile_hook = get_axon_ntff_profile_hook
        sys.modules["antenv.axon_hooks"] = mod
        try:
            import antenv

            antenv.axon_hooks = mod
        except Exception:
            pass
    m = sys.modules["antenv.axon_hooks"]
    if m._hook is None:
        try:
            from trn_agent_boot.trn_boot import _ntff_profile_via_ctypes

            m.set_axon_ntff_profile_hook(
                _ntff_profile_via_ctypes("/opt/axon/libaxon_pjrt.so")
            )
        except Exception as e:
            print(f"ntff hook install failed: {e}")
    import concourse.bass_utils as bu

    if not getattr(bu, "_upload_patched", False):
        bu.upload_artifacts = lambda tmpdir: f"local:{tmpdir}"
        bu._upload_patched = True


def kernel_with_results(feats, mask, transitions, trace=False):
    from concourse.bass_utils import run_bass_kernel_spmd

    if trace:
        _install_trace_support()

    feats = np.asarray(feats, dtype=np.float32)
    mask = np.asarray(mask, dtype=np.float32)
    S, Btot, Tt = feats.shape
    assert Tt == T and Btot % NCORES == 0
    b = Btot // NCORES
    assert b == B

    lens = mask.sum(axis=0)
    kf = 16
    mask_from = int(min(lens.min() // kf * kf, S))
    nc = _get_nc(S, mask_from)

    small = _prep_small_inputs(transitions)
    in_maps = []
    for c in range(NCORES):
        sl = slice(c * b, (c + 1) * b)
        in_maps.append(
            {
                "feats": np.ascontiguousarray(feats[:, sl, :]),
                "mask": np.ascontiguousarray(mask[:, sl]),
                **small,
            }
        )

    import tempfile

    tmpdir = tempfile.mkdtemp(prefix="crf_trace_") if trace else None
    res = run_bass_kernel_spmd(
        nc, in_maps, list(range(NCORES)), trace=trace, tmpdir=tmpdir
    )
    if trace:
        print(f"trace dir: {tmpdir}")
    outs = res.results
    logZ = np.concatenate([np.asarray(o["logZ"]).reshape(-1) for o in outs])
    best = np.concatenate([np.asarray(o["best_score"]).reshape(-1) for o in outs])
    ptr = np.concatenate(
        [np.asarray(o["pointers"]).reshape(S, b, T) for o in outs], axis=1
    )
    return (logZ.astype(np.float32), best.astype(np.float32), ptr.astype(np.int32)), res


def kernel(feats, mask, transitions):
    (logZ, best, ptr), _ = kernel_with_results(feats, mask, transitions, trace=False)
    return logZ, best, ptr


# revision 31
# speedup vs baseline: 1.4261x; 1.0068x over previous
"""BiLSTM-CRF forward+Viterbi Trainium2 kernel (8-core data-parallel).

Computes, for feats [S,B,T] f32, mask [S,B] f32, transitions [T,T] f32:
  logZ [B] f32         -- CRF forward log-partition
  best_score [B] f32   -- Viterbi max score
  pointers [S,B,T] i32 -- Viterbi argmax backpointers (first-max ties, exact)

Strategy (per core, B_loc=128 batches on SBUF partitions):
  Viterbi (exact fp32, bitwise-matching the jax reference):
    upd[b,i,j] = s[b,j] + trans[i,j]        (DVE tensor_tensor, broadcast AP)
    best = segmented reduce_max over j      (DVE tensor_reduce axis=X)
    wsel = (upd >= best) * (31-j)           (custom DVE op, 8-step batched)
    maxw = segmented reduce_max over j      -> ptr = 31 - maxw (ScalarE, i32 out)
    s    = mask ? best+feat : s             (DVE add + copy_predicated)
  The wsel/maxw extraction reproduces jnp.argmax first-max tie semantics
  exactly (best is the reduce_max of the same fl(s+t) values).
  Forward in linear space u = C*exp(alpha), tag-major [T,128]:
    v = exp(trans) @ u                      (PE matmul)
    u = mask ? v*exp(feat) : u              (DVE mult + copy_predicated)
    renorm every 8 steps: u /= sum(u); logacc += log(sum)   (PE+DVE+ACT)
  feats are DMA-streamed in 16-step chunks; transposed on PE for the
  tag-major exp(feat); pointers accumulated per chunk and DMA'd out.
"""

import os
import numpy as np

S_FULL, B_TOT, T, NCORES = 1024, 1024, 32, 8
B = B_TOT // NCORES
START_TAG, STOP_TAG, PAD_TAG, NEG_INF = 29, 30, 31, -10000.0

_NC_CACHE = {}
_CUSTOM_OP = None


def _get_custom_op():
    """Register SEG_ARGMAX_W_ANT: out = (in0 >= in1) * ((c0 - Idx) + c1*SubIdx).

    With in0 = upd [P, (g i) pages, j inner], c0=31, c1=32(=j count): the g/i
    page terms cancel and out = (upd >= best) * (31 - j) for every step g in
    the batch. A reduce_max over j then yields 31 - argmax_j with first-max
    (lowest-j) tie semantics, matching jnp.argmax exactly.
    """
    global _CUSTOM_OP
    if _CUSTOM_OP is not None:
        return _CUSTOM_OP
    import concourse.dve_ops as dops
    from concourse.dve_spec import Spec, Src0, Src1, C0, C1, SubIdx, Idx, lower
    from concourse.dve_uop import DveOpSpec

    name = "SEG_ARGMAX_W_ANT"
    if name in dops.CUSTOM_DVE_SPECS:
        _CUSTOM_OP = next(o for o in dops.OPS if o.name == name)
        return _CUSTOM_OP

    def _ref(in0, in1, c0, c1, c2):
        x = np.asarray(in0, dtype=np.float32)
        y = np.broadcast_to(np.asarray(in1, dtype=np.float32), x.shape)
        P = x.shape[0]
        N = x.shape[-1]
        flat = x.reshape(P, -1)
        n = flat.shape[1]
        idx = np.arange(n, dtype=np.float32)
        page = np.float32(np.arange(n) // N)
        w = (np.float32(c0) - idx) + np.float32(c1) * page
        out = (flat >= y.reshape(P, -1)).astype(np.float32) * w[None, :]
        return out.reshape(x.shape).astype(np.float32)

    spec = Spec(body=(Src0 >= Src1) * ((C0 - Idx) + C1 * SubIdx), reference=_ref)
    row = dops._CUSTOM_DVE_ROW_BASE + len(dops.OPS)
    assert row < 0x20, "custom DVE row table full"
    shas = {}
    for ver in ("v3", "v4"):
        uops = lower(spec, ver=ver)
        shas[ver] = DveOpSpec(name=name, opcode=row, uops=uops, rd1_en=True).sha(ver)
    op = dops.DveOp(name, spec, subdim=True, uops_sha=shas)
    dops.OPS.append(op)
    dops.CUSTOM_DVE_SPECS[name] = spec
    dops._SUB_OPCODE_FOR_NAME[name] = row
    _CUSTOM_OP = op
    return op


def build_nc(S, mask_from, kf=16, renorm=8, b=B, use_custom=True):
    """Build the per-core Bass program (identical on all cores)."""
    import concourse.bass as bass
    import concourse.bacc as bacc
    import concourse.mybir as mybir
    import concourse.tile as tile
    from concourse.mybir import AluOpType as alu
    from contextlib import ExitStack

    f32 = mybir.dt.float32
    bf16 = mybir.dt.bfloat16
    i32 = mybir.dt.int32
    AX = mybir.AxisListType
    AF = mybir.ActivationFunctionType

    assert S % kf == 0 and mask_from % kf == 0
    assert kf % 4 == 0

    cop = _get_custom_op() if use_custom else None

    nc = bacc.Bacc("TRN2", target_bir_lowering=False, debug=False)

    feats_d = nc.declare_dram_parameter("feats", [S, b, T], f32, isOutput=False)
    mask_d = nc.declare_dram_parameter("mask", [S, b], f32, isOutput=False)
    transr_d = nc.declare_dram_parameter("trans_rep", [128, T * T], f32, isOutput=False)
    transTe_d = nc.declare_dram_parameter("transT_exp", [T, T], f32, isOutput=False)
    tstopr_d = nc.declare_dram_parameter("tstop_rep", [128, T], f32, isOutput=False)
    estop_d = nc.declare_dram_parameter("estop_col", [T, 1], f32, isOutput=False)
    wrep_d = nc.declare_dram_parameter("w_rep", [128, T * T], f32, isOutput=False)
    u0_d = nc.declare_dram_parameter("u0", [T, 128], f32, isOutput=False)
    ident_d = nc.declare_dram_parameter("ident128", [128, 128], f32, isOutput=False)

    logz_d = nc.declare_dram_parameter("logZ", [b], f32, isOutput=True)
    best_d = nc.declare_dram_parameter("best_score", [b], f32, isOutput=True)
    ptr_d = nc.declare_dram_parameter("pointers", [S, b, T], i32, isOutput=True)

    with tile.TileContext(nc) as tc, ExitStack() as ctx:
        # ---------------- pools ----------------
        consts = ctx.enter_context(tc.tile_pool(name="consts", bufs=1))
        state = ctx.enter_context(tc.tile_pool(name="state", bufs=1))
        fpool = ctx.enter_context(tc.tile_pool(name="fpool", bufs=2))
        ppool = ctx.enter_context(tc.tile_pool(name="ppool", bufs=2))
        vit = ctx.enter_context(tc.tile_pool(name="vit", bufs=3))
        fwd = ctx.enter_context(tc.tile_pool(name="fwd", bufs=3))
        mrow = ctx.enter_context(tc.tile_pool(name="mrow", bufs=2))
        ps_v = ctx.enter_context(tc.tile_pool(name="ps_v", bufs=2, space="PSUM"))
        ps_tr = ctx.enter_context(tc.tile_pool(name="ps_tr", bufs=2, space="PSUM"))
        ps_m = ctx.enter_context(tc.tile_pool(name="ps_m", bufs=2, space="PSUM"))
        ps_s = ctx.enter_context(tc.tile_pool(name="ps_s", bufs=1, space="PSUM"))
        ps_r = ctx.enter_context(tc.tile_pool(name="ps_r", bufs=1, space="PSUM"))

        # ---------------- constants ----------------
        ident = consts.tile([128, 128], f32)
        nc.sync.dma_start(ident[:], ident_d.ap())

        trans_rep = consts.tile([128, T, T], f32)
        nc.sync.dma_start(trans_rep[:], transr_d.ap())

        if not use_custom:
            w_rep_f = consts.tile([128, T, T], f32)
            nc.sync.dma_start(w_rep_f[:], wrep_d.ap())
            w_rep = consts.tile([128, T, T], bf16)
            nc.vector.tensor_copy(w_rep[:], w_rep_f[:])

        tstop_rep = consts.tile([128, T], f32)
        nc.sync.dma_start(tstop_rep[:], tstopr_d.ap())

        et_sb = consts.tile([T, T], f32)
        nc.sync.dma_start(et_sb[:], transTe_d.ap())
        estop_sb = consts.tile([T, 1], f32)
        nc.sync.dma_start(estop_sb[:], estop_d.ap())

        ones_1xT = consts.tile([1, T], f32)
        nc.vector.memset(ones_1xT[:], 1.0)
        ones_Tx1 = consts.tile([T, 1], f32)
        nc.vector.memset(ones_Tx1[:], 1.0)

        # ---------------- persistent state ----------------
        s_bm = state.tile([128, T], f32)  # Viterbi scores, batch-major
        nc.vector.memset(s_bm[:], NEG_INF)
        nc.vector.memset(s_bm[:, START_TAG : START_TAG + 1], 0.0)

        u_tm = state.tile([T, 128], f32)  # forward linear state, tag-major
        nc.sync.dma_start(u_tm[:], u0_d.ap())

        logacc = state.tile([1, 128], f32)
        nc.vector.memset(logacc[:], 0.0)

        maskT_sb = state.tile([128, 128], f32)  # [b, s-within-128-chunk]

        # ---------------- main loop ----------------
        n_chunks = S // kf
        for c in range(n_chunks):
            s0 = c * kf
            masked = s0 >= mask_from

            feats_bm = fpool.tile([128, kf, T], f32, name=f"feats_bm_{c}", tag="feats_bm")
            nc.sync.dma_start(feats_bm[:], feats_d.ap()[s0 : s0 + kf, :, :].transpose([1, 0, 2]))

            ptrc = ppool.tile([128, kf, T], i32, name=f"ptrc_{c}", tag="ptrc")

            if s0 % 128 == 0 and s0 + 128 > mask_from:
                # batch-major mask block for the next up-to-128 steps
                blk = min(128, S - s0)
                mk_sp = mrow.tile([128, 128], f32, name=f"mk_sp_{c}", tag="mk_sp")
                nc.sync.dma_start(mk_sp[:blk, :], mask_d.ap()[s0 : s0 + blk, :])
                mk_ps = ps_tr.tile([128, 128], f32, name=f"mk_ps_{c}", tag="tr128")
                nc.tensor.transpose(mk_ps[:, :blk], mk_sp[:blk, :], ident[:blk, :blk])
                nc.scalar.copy(maskT_sb[:, :blk], mk_ps[:, :blk])
            if masked:
                maskrow = mrow.tile([1, kf, 128], f32, name=f"maskrow_{c}", tag="maskrow")
                nc.sync.dma_start(maskrow[:], mask_d.ap()[s0 : s0 + kf, :].unsqueeze(0))

            for q in range(kf // 4):
                # transpose 4 steps of feats to tag-major, then exp -> g4
                ftr_ps = ps_tr.tile([128, 128], f32, name=f"ftr_{c}_{q}", tag="tr128")
                nc.tensor.transpose(ftr_ps[:], feats_bm[:, 4 * q : 4 * q + 4, :], ident[:])
                g4 = fwd.tile([128, 128], f32, name=f"g4_{c}_{q}", tag="g4")
                nc.scalar.activation(g4[:], ftr_ps[:], AF.Exp)

                mask_tm = None
                if masked:
                    mask_tm = ps_m.tile([T, 4, 128], f32, name=f"mask_tm_{c}_{q}", tag="mask_tm")
                    nc.tensor.matmul(
                        mask_tm[:], ones_1xT[:], maskrow[:, 4 * q : 4 * q + 4, :]
                    )

                if use_custom and q % 2 == 0:
                    upd8 = vit.tile([128, 8, T, T], f32, name=f"upd8_{c}_{q}", tag="upd8")
                    best8 = vit.tile([128, 8, T], f32, name=f"best8_{c}_{q}", tag="best8")

                for dt in range(4):
                    t = s0 + 4 * q + dt
                    feat_t = feats_bm[:, 4 * q + dt, :]

                    # ---- Viterbi ----
                    if use_custom:
                        g8 = 4 * (q % 2) + dt
                        upd = upd8[:, g8]
                        bestt = best8[:, g8]
                    else:
                        upd = vit.tile([128, T, T], f32, name=f"upd_{t}", tag="upd")[:]
                        bestt = vit.tile([128, T], f32, name=f"best_{t}", tag="best")[:]
                    nc.vector.tensor_tensor(
                        upd,
                        s_bm[:].unsqueeze(1).broadcast_to([128, T, T]),
                        trans_rep[:],
                        alu.add,
                    )
                    nc.vector.reduce_max(bestt, upd, axis=AX.X)

                    if not use_custom:
                        eq = vit.tile([128, T, T], bf16, name=f"eq_{t}", tag="eq")
                        nc.vector.tensor_tensor(
                            eq[:],
                            upd,
                            bestt.unsqueeze(2).broadcast_to([128, T, T]),
                            alu.is_ge,
                        )
                        wsl = vit.tile([128, T, T], bf16, name=f"wsl_{t}", tag="wsl")
                        nc.vector.tensor_tensor(wsl[:], eq[:], w_rep[:], alu.mult)
                        maxw = vit.tile([128, T], f32, name=f"maxw_{t}", tag="maxw")
                        nc.vector.reduce_max(maxw[:], wsl[:], axis=AX.X)
                        # ptr = 31 - maxw, cast to i32 (ScalarE)
                        nc.scalar.activation(
                            ptrc[:, 4 * q + dt, :], maxw[:], AF.Copy, bias=31.0, scale=-1.0
                        )

                    # ---- scores update ----
                    if not masked:
                        nc.vector.tensor_tensor(s_bm[:], bestt, feat_t, alu.add)
                    else:
                        upd_s = vit.tile([128, T], f32, name=f"upds_{t}", tag="upds")
                        nc.vector.tensor_tensor(upd_s[:], bestt, feat_t, alu.add)
                        mcol = (
                            maskT_sb[:, t % 128 : t % 128 + 1]
                            .bitcast(mybir.dt.int32)
                            .broadcast_to([128, T])
                        )
                        nc.vector.copy_predicated(s_bm[:], mcol, upd_s[:])

                    # ---- forward ----
                    vps = ps_v.tile([T, 128], f32, name=f"v_{t}", tag="v")
                    nc.tensor.matmul(vps[:], et_sb[:], u_tm[:])
                    g_t = g4[32 * dt : 32 * dt + 32, :]
                    if not masked:
                        nc.vector.tensor_tensor(u_tm[:], vps[:], g_t, alu.mult)
                    else:
                        unew = fwd.tile([T, 128], f32, name=f"unew_{t}", tag="unew")
                        nc.vector.tensor_tensor(unew[:], vps[:], g_t, alu.mult)
                        nc.vector.copy_predicated(
                            u_tm[:], mask_tm[:, dt, :].bitcast(mybir.dt.int32), unew[:]
                        )

                    # ---- renorm ----
                    if (t + 1) % renorm == 0:
                        sb_ps = ps_s.tile([1, 128], f32, name=f"sb_{t}", tag="sb")
                        nc.tensor.matmul(sb_ps[:], ones_Tx1[:], u_tm[:])
                        lg = fwd.tile([1, 128], f32, name=f"lg_{t}", tag="lg")
                        nc.scalar.activation(lg[:], sb_ps[:], AF.Ln)
                        # 1/sb = exp(-lg) on ScalarE (keeps reciprocal off DVE);
                        # LUT error cancels in logZ to ~1e-4 absolute.
                        recip = fwd.tile([1, 128], f32, name=f"recip_{t}", tag="recip")
                        nc.scalar.activation(recip[:], lg[:], AF.Exp, scale=-1.0)
                        nc.vector.tensor_tensor(logacc[:], logacc[:], lg[:], alu.add)
                        rb_ps = ps_r.tile([T, 128], f32, name=f"rb_{t}", tag="rb")
                        nc.tensor.matmul(rb_ps[:], ones_1xT[:], recip[:])
                        nc.vector.tensor_tensor(u_tm[:], u_tm[:], rb_ps[:], alu.mult)

                if use_custom and q % 2 == 1:
                    # fused (upd >= best) * (31 - j) over the 8-step batch.
                    # wsel values are 0..31 integers -> bf16-exact, so the
                    # segmented max runs as a contiguous-halves tensor_tensor
                    # max tree in bf16 2x mode (tensor_reduce is 1x-only).
                    wsl8 = vit.tile(
                        [128, 8, T, T], bf16, name=f"wsl8_{c}_{q}", tag="wsl8", bufs=1
                    )
                    nc.vector._custom_dve(
                        cop,
                        out=wsl8[:].rearrange("p g a b -> p (g a) b"),
                        in0=upd8[:].rearrange("p g a b -> p (g a) b"),
                        in1=best8[:]
                        .rearrange("p g a -> p (g a)")
                        .unsqueeze(2)
                        .broadcast_to([128, 8 * T, T]),
                        s0=31.0,
                        s1=float(T),
                    )
                    w3 = wsl8[:].rearrange("p g a b -> p (g a) b")  # [128, 256, 32]
                    m1 = vit.tile([128, 256, 16], bf16, name=f"m1_{c}_{q}", tag="m1", bufs=1)
                    nc.vector.tensor_tensor(m1[:], w3[:, :, 0:16], w3[:, :, 16:32], alu.max)
                    m2 = vit.tile([128, 256, 8], bf16, name=f"m2_{c}_{q}", tag="m2", bufs=1)
                    nc.vector.tensor_tensor(m2[:], m1[:, :, 0:8], m1[:, :, 8:16], alu.max)
                    m3 = vit.tile([128, 256, 4], bf16, name=f"m3_{c}_{q}", tag="m3", bufs=1)
                    nc.vector.tensor_tensor(m3[:], m2[:, :, 0:4], m2[:, :, 4:8], alu.max)
                    m4 = vit.tile([128, 256, 2], bf16, name=f"m4_{c}_{q}", tag="m4", bufs=1)
                    nc.vector.tensor_tensor(m4[:], m3[:, :, 0:2], m3[:, :, 2:4], alu.max)
                    maxw8 = vit.tile([128, 8, T], bf16, name=f"maxw8_{c}_{q}", tag="maxw8")
                    nc.vector.tensor_tensor(
                        maxw8[:].rearrange("p g a -> p (g a)").unsqueeze(2),
                        m4[:, :, 0:1],
                        m4[:, :, 1:2],
                        alu.max,
                    )
                    nc.scalar.activation(
                        ptrc[:, 4 * q - 4 : 4 * q + 4, :],
                        maxw8[:],
                        AF.Copy,
                        bias=31.0,
                        scale=-1.0,
                    )

            nc.sync.dma_start(
                ptr_d.ap()[s0 : s0 + kf, :, :].transpose([1, 0, 2]), ptrc[:]
            )

        # ---------------- endgame ----------------
        zps = ps_s.tile([1, 128], f32, name="zps", tag="sb")
        nc.tensor.matmul(zps[:], estop_sb[:], u_tm[:])
        lnz = fwd.tile([1, 128], f32, name="lnz", tag="lg")
        nc.scalar.activation(lnz[:], zps[:], AF.Ln)
        logz_t = fwd.tile([1, 128], f32, name="logz_t", tag="recip")
        nc.vector.tensor_tensor(logz_t[:], logacc[:], lnz[:], alu.add)
        nc.sync.dma_start(logz_d.ap().unsqueeze(0), logz_t[:])

        sc = vit.tile([128, T], f32, name="sc", tag="upds")
        nc.vector.tensor_tensor(sc[:], s_bm[:], tstop_rep[:], alu.add)
        bsc = vit.tile([128, 1], f32, name="bsc", tag="maxw")
        nc.vector.reduce_max(bsc[:], sc[:], axis=AX.X)
        nc.sync.dma_start(best_d.ap().unsqueeze(1), bsc[:])

    nc.compile()
    return nc


def _prep_small_inputs(transitions):
    f32 = np.float32
    tr = np.asarray(transitions, dtype=f32)
    trans_rep = np.ascontiguousarray(
        np.broadcast_to(tr.reshape(1, T * T), (128, T * T))
    )
    e = np.exp(np.maximum(tr, f32(-87.0))).astype(f32)
    transT_exp = np.ascontiguousarray(e.T)
    tstop_rep = np.ascontiguousarray(np.broadcast_to(tr[STOP_TAG], (128, T)))
    estop_col = np.ascontiguousarray(e[STOP_TAG].reshape(T, 1))
    w_rep = np.ascontiguousarray(
        np.broadcast_to(
            np.tile((31 - np.arange(T)).astype(f32), T).reshape(1, T * T),
            (128, T * T),
        )
    )
    u0 = np.zeros((T, 128), dtype=f32)
    u0[START_TAG, :] = 1.0
    return {
        "u0": u0,
        "trans_rep": trans_rep,
        "transT_exp": transT_exp,
        "tstop_rep": tstop_rep,
        "estop_col": estop_col,
        "w_rep": w_rep,
        "ident128": np.eye(128, dtype=f32),
    }


def _get_nc(S, mask_from):
    use_custom = os.environ.get("CRF_NO_CUSTOM", "0") != "1"
    key = (S, mask_from, use_custom)
    if key not in _NC_CACHE:
        _NC_CACHE[key] = build_nc(S, mask_from, use_custom=use_custom)
    return _NC_CACHE[key]


def _install_trace_support():
    """Synthesize the missing antenv.axon_hooks module + disable artifact upload."""
    import sys, types

    if "antenv.axon_hooks" not in sys.modules:
        mod = types.ModuleType("antenv.axon_hooks")
        mod._hook = None

        def set_axon_ntff_profile_hook(h):
            mod._hook = h

        def get_axon_ntff_profile_hook():
            return mod._hook

        mod.set_axon_ntff_profile_hook = set_axon_ntff_profile_hook
        mod.get_axon_ntff_profile_hook = get_axon_ntff_profile_hook
        sys.modules["antenv.axon_hooks"] = mod
        try:
            import antenv

            antenv.axon_hooks = mod
        except Exception:
            pass
    m = sys.modules["antenv.axon_hooks"]
    if m._hook is None:
        try:
            from trn_agent_boot.trn_boot import _ntff_profile_via_ctypes

            m.set_axon_ntff_profile_hook(
                _ntff_profile_via_ctypes("/opt/axon/libaxon_pjrt.so")
            )
        except Exception as e:
            print(f"ntff hook install failed: {e}")
    import concourse.bass_utils as bu

    if not getattr(bu, "_upload_patched", False):
        bu.upload_artifacts = lambda tmpdir: f"local:{tmpdir}"
        bu._upload_patched = True


def kernel_with_results(feats, mask, transitions, trace=False):
    from concourse.bass_utils import run_bass_kernel_spmd

    if trace:
        _install_trace_support()

    feats = np.asarray(feats, dtype=np.float32)
    mask = np.asarray(mask, dtype=np.float32)
    S, Btot, Tt = feats.shape
    assert Tt == T and Btot % NCORES == 0
    b = Btot // NCORES
    assert b == B

    lens = mask.sum(axis=0)
    kf = 16
    mask_from = int(min(lens.min() // kf * kf, S))
    nc = _get_nc(S, mask_from)

    small = _prep_small_inputs(transitions)
    in_maps = []
    for c in range(NCORES):
        sl = slice(c * b, (c + 1) * b)
        in_maps.append(
            {
                "feats": np.ascontiguousarray(feats[:, sl, :]),
                "mask": np.ascontiguousarray(mask[:, sl]),
                **small,
            }
        )

    import tempfile

    tmpdir = tempfile.mkdtemp(prefix="crf_trace_") if trace else None
    res = run_bass_kernel_spmd(
        nc, in_maps, list(range(NCORES)), trace=trace, tmpdir=tmpdir
    )
    if trace:
        print(f"trace dir: {tmpdir}")
    outs = res.results
    logZ = np.concatenate([np.asarray(o["logZ"]).reshape(-1) for o in outs])
    best = np.concatenate([np.asarray(o["best_score"]).reshape(-1) for o in outs])
    ptr = np.concatenate(
        [np.asarray(o["pointers"]).reshape(S, b, T) for o in outs], axis=1
    )
    return (logZ.astype(np.float32), best.astype(np.float32), ptr.astype(np.int32)), res


def kernel(feats, mask, transitions):
    (logZ, best, ptr), _ = kernel_with_results(feats, mask, transitions, trace=False)
    return logZ, best, ptr
